# revision 2
# baseline (speedup 1.0000x reference)
"""Trainium2 Bass kernel v3 for nn_ClassificationModel.

Data parallel across 8 NeuronCores: batch N=64 -> 8 samples/core.

Differences vs v2 baseline:
  - All per-layer transformer weights packed into ONE [128, 13824] bf16 DMA
    (one HWDGE acquire per layer instead of ~40), double-buffered prefetch.
  - Scores read compact feature-major Q/K via partition-offset lhsT slices
    (heads 2/5 split across chunk boundaries -> 2 accumulating matmuls);
    the per-head 64-row spread DMAs are gone entirely.
  - LayerNorm affine (g, be) folded into adjacent weights host-side; device
    state is the *normalized* activation xn in bf16 (row- and feature-major).
    Residual enters the pre-LN sum as a single identity matmul.  LN stats
    come from accum_out sums (copy + square), apply is one tensor_scalar.
  - 2-PSUM-bank (free=1024) elementwise units for QK bias, scores exp and
    FFN relu; PSUM-consuming ops rotated across ACT/DVE/Pool for balance.
"""

import math
import sys

sys.path.insert(0, "/opt/trn_rl_repo")

import numpy as np
import ml_dtypes

import concourse.bass as bass
import concourse.mybir as mybir
import concourse.tile as tile
from concourse import bacc
from concourse.bass import AP
from concourse.bass_utils import run_bass_kernel_spmd

BF = ml_dtypes.bfloat16
F32 = mybir.dt.float32
BF16 = mybir.dt.bfloat16
AX = mybir.AxisListType
OP = mybir.AluOpType
AF = mybir.ActivationFunctionType

# model dims
N, L, W = 64, 128, 256
D, H, NL, DFF = 384, 8, 4, 1536
E = D // H  # 48
CH = [1, 4, 16, 64]
K = 7
NCORES = 8
RPC = N // NCORES          # samples per core = 8
R = RPC * L                # rows per core = 1024
TEMP = 1.0 / math.sqrt(E)
EPS = 1e-5

# conv block sizes (output positions per Toeplitz block)
B0, B1, B2 = 32, 8, 2
NB0, NB1, NB2 = 256 // B0, 128 // B1, 64 // B2  # 8, 16, 32

# packed per-layer weight blob column offsets (bf16, [128, WCOLS])
# chunk c in 0..2: Wq(512, head-padded) Wk(512) Wv(384) Wo(384) W1(1536)
# at c*3328; then W2: 12 chunks x 384 at 9984.
# Q/K output features are head-padded: head h -> rows 64h..64h+47 of 512,
# so per-head score matmuls read base partitions 0/64 (hw constraint).
CSEC = 512 + 512 + 384 + 384 + 1536  # 3328
WCOLS = 3 * CSEC + 12 * 384          # 14592


# ---------------------------------------------------------------------------
# host-side weight preparation
# ---------------------------------------------------------------------------

def _pe_np(l, d):
    pos = np.arange(l)[:, None].astype(np.float32)
    i = np.arange(d // 2)[None, :].astype(np.float32)
    ang = pos / np.power(10000.0, 2.0 * i / d)
    pe = np.zeros((l, d), np.float32)
    pe[:, 0::2] = np.sin(ang)
    pe[:, 1::2] = np.cos(ang)
    return pe


# conv source-block overlap enumeration (shared host/device) -----------------

CONV_GEOM = {
    0: (B0, 128, 2, 1),
    1: (B1, 16, NB0, 4),
    2: (B2, 4, NB1, 16),
}


def overlaps(conv, b):
    Bout, src_size, nsrc, _ = CONV_GEOM[conv]
    w0, w1 = Bout * b - 3, Bout * b + Bout + 3
    res = []
    for s in range(nsrc):
        lo, hi = s * src_size, (s + 1) * src_size
        if max(w0, lo) < min(w1, hi):
            res.append((s, lo - Bout * b))
    return res


def conv_deltas(conv):
    nb = {0: NB0, 1: NB1, 2: NB2}[conv]
    return sorted({d for b in range(nb) for _, d in overlaps(conv, b)})


def _m_layout(conv, h, co):
    if conv == 0:
        return (h & 1) * 64 + (h >> 1) * 4 + co
    if conv == 1:
        return (h & 1) * 64 + (h >> 1) * 16 + co
    return h * 64 + co


def _toeplitz_variants(conv, w):
    Bout, src_size, _, nch = CONV_GEOM[conv]
    cout = w.shape[0]
    ds = conv_deltas(conv)
    T = np.zeros((len(ds), src_size * nch, 128), np.float32)
    for vi, delta in enumerate(ds):
        for hp in range(src_size):
            for h in range(Bout):
                k = delta + hp - h + 3
                if 0 <= k < K:
                    for co in range(cout):
                        for ci in range(nch):
                            T[vi, hp * nch + ci, _m_layout(conv, h, co)] = w[co, ci, k]
    return T


def host_prep(inp):
    d = {}
    f32 = np.float32
    d["T0"] = _toeplitz_variants(0, np.asarray(inp["conv_w0"], f32)).astype(BF)
    d["T1"] = _toeplitz_variants(1, np.asarray(inp["conv_w1"], f32)).astype(BF)
    d["T2"] = _toeplitz_variants(2, np.asarray(inp["conv_w2"], f32)).astype(BF)
    b0, b1c, b2c = (np.asarray(inp[f"conv_b{i}"], f32) for i in range(3))
    p = np.arange(128)
    d["b0e"] = b0[p % 4].reshape(128, 1)
    d["b1e"] = b1c[p % 16].reshape(128, 1)
    d["b2e"] = b2c[p % 64].reshape(128, 1)

    # embed: We_r[c, p, :] = embed_w[(p%64)*32 + 2c + p//64, :]
    ew = np.asarray(inp["embed_w"], f32)  # (2048, 384)
    We_r = np.zeros((16, 128, D), f32)
    for c in range(16):
        for pi in range(128):
            We_r[c, pi] = ew[(pi % 64) * 32 + 2 * c + pi // 64]
    d["We_r"] = We_r.astype(BF)
    d["eb_row"] = np.asarray(inp["embed_b"], f32).reshape(1, D).astype(BF)
    d["pe_rm"] = _pe_np(L, D)

    g1 = np.asarray(inp["g1"], f32)
    be1 = np.asarray(inp["be1"], f32)
    g2 = np.asarray(inp["g2"], f32)
    be2 = np.asarray(inp["be2"], f32)

    # pending affine entering each layer's attention block
    gp = np.stack([np.ones(D, f32) if l == 0 else g2[l - 1] for l in range(NL)])
    bp = np.stack([np.zeros(D, f32) if l == 0 else be2[l - 1] for l in range(NL)])
    d["gp_identity"] = [bool(np.all(gp[l] == 1.0)) for l in range(NL)] + \
                       [bool(np.all(g1[l] == 1.0)) for l in range(NL)]

    Wq = np.asarray(inp["Wq"], f32)
    Wk = np.asarray(inp["Wk"], f32)
    Wv = np.asarray(inp["Wv"], f32)
    Wo = np.asarray(inp["Wo"], f32)
    W1 = np.asarray(inp["W1"], f32)
    W2 = np.asarray(inp["W2"], f32)
    bq = np.asarray(inp["bq"], f32)
    bk = np.asarray(inp["bk"], f32)
    bv = np.asarray(inp["bv"], f32)
    bo = np.asarray(inp["bo"], f32)
    b1 = np.asarray(inp["b1"], f32)
    b2 = np.asarray(inp["b2"], f32)

    # fold pending affines into weights/biases
    WqF = gp[:, :, None] * Wq
    WkF = gp[:, :, None] * Wk
    WvF = gp[:, :, None] * Wv
    bqF = bq + np.einsum("ld,lde->le", bp, Wq)
    bkF = bk + np.einsum("ld,lde->le", bp, Wk)
    bvF = bv + np.einsum("ld,lde->le", bp, Wv)
    W1F = g1[:, :, None] * W1
    b1F = b1 + np.einsum("ld,lde->le", be1, W1)
    boF = bo + bp          # LN1 pre-sum bias includes pending be
    b2F = b2 + be1         # LN2 pre-sum bias includes LN1's be

    # diag blocks for non-identity pending g (3 chunks of [128, 384] each)
    def diag_chunks(g):
        out = np.zeros((128, 3 * D), f32)
        for c in range(3):
            for i in range(128):
                out[i, c * D + c * 128 + i] = g[c * 128 + i]
        return out
    d["dg_attn"] = np.stack([diag_chunks(gp[l]) for l in range(NL)]).astype(BF)
    d["dg_ffn"] = np.stack([diag_chunks(g1[l]) for l in range(NL)]).astype(BF)

    # head-pad Q/K output features: head h -> cols 64h..64h+47 of 512
    def head_pad_w(w):  # (NL, 384, 384) -> (NL, 384, 512)
        out = np.zeros((NL, D, 512), f32)
        for h in range(H):
            out[:, :, 64 * h:64 * h + E] = w[:, :, E * h:E * (h + 1)]
        return out

    def head_pad_b(b):  # (NL, 384) -> (NL, 512)
        out = np.zeros((NL, 512), f32)
        for h in range(H):
            out[:, 64 * h:64 * h + E] = b[:, E * h:E * (h + 1)]
        return out

    WqP, WkP = head_pad_w(WqF), head_pad_w(WkF)
    bqP, bkP = head_pad_b(bqF), head_pad_b(bkF)

    # mega weight blob per layer
    WL = np.zeros((NL, 128, WCOLS), f32)
    for l in range(NL):
        for c in range(3):
            r = slice(c * 128, (c + 1) * 128)
            base = c * CSEC
            WL[l, :, base + 0:base + 512] = WqP[l][r]
            WL[l, :, base + 512:base + 1024] = WkP[l][r]
            WL[l, :, base + 1024:base + 1408] = WvF[l][r]
            WL[l, :, base + 1408:base + 1792] = Wo[l][r]
            WL[l, :, base + 1792:base + 3328] = W1F[l][r]
        for dc in range(12):
            WL[l, :, 3 * CSEC + dc * 384:3 * CSEC + (dc + 1) * 384] = \
                W2[l][dc * 128:(dc + 1) * 128]
    d["WL"] = WL.astype(BF)

    # per-layer f32 bias blob [128, 20]: bq (4 padded chunks), bk, b1r (12)
    BL = np.zeros((NL, 128, 20), f32)
    for oc in range(4):
        BL[:, :, oc] = bqP[:, oc * 128:(oc + 1) * 128]
        BL[:, :, 4 + oc] = bkP[:, oc * 128:(oc + 1) * 128]
    for l in range(NL):
        BL[l, :, 8:20] = b1F[l].reshape(12, 128).T
    d["BL"] = BL

    # per-layer bf16 rows blob [1, 1152]: bv | bo' | b2'
    RL = np.zeros((NL, 1, 3 * D), f32)
    RL[:, 0, 0:D] = bvF
    RL[:, 0, D:2 * D] = boF
    RL[:, 0, 2 * D:3 * D] = b2F
    d["RL"] = RL.astype(BF)

    d["idn_f"] = np.eye(128, dtype=f32)
    d["idn_b"] = np.eye(128, dtype=f32).astype(BF)
    d["onesL"] = np.full((128, 1), 1.0 / L, f32).astype(BF)
    # head fold: mean(g2[3]*xn + be2[3]) @ cls_w + cls_b
    cw = np.asarray(inp["cls_w"], f32)          # (384, 1)
    cb = np.asarray(inp["cls_b"], f32)          # (1,)
    cwF = (g2[NL - 1][:, None] * cw)
    cbF = cb + be2[NL - 1] @ cw
    d["clsw_r"] = cwF.reshape(3, 128).T.copy()  # (128, 3)
    d["clsb"] = cbF.reshape(1, 1)
    d["epsc"] = np.full((128, 1), EPS, f32)
    return d


# ---------------------------------------------------------------------------
# device program
# ---------------------------------------------------------------------------

def build_program(gp_ident=None, do_compile=True, n_layers=NL, phase=99, split_exp=False, pad_scores=False, even_only=False):
    if gp_ident is None:
        gp_ident = [True] * (2 * NL)
    nc = bacc.Bacc("TRN2", target_bir_lowering=False, debug=False)

    def dram_in(name, shape, dt=BF16):
        return nc.dram_tensor(name, list(shape), dt, kind="ExternalInput")

    x_d = dram_in("xc", (128, RPC, W), F32)
    nv0, nv1, nv2 = len(conv_deltas(0)), len(conv_deltas(1)), len(conv_deltas(2))
    T0_d = dram_in("T0", (nv0, 128, 128))
    T1_d = dram_in("T1", (nv1, 64, 128))
    T2_d = dram_in("T2", (nv2, 64, 128))
    b0e_d = dram_in("b0e", (128, 1), F32)
    b1e_d = dram_in("b1e", (128, 1), F32)
    b2e_d = dram_in("b2e", (128, 1), F32)
    We_d = dram_in("We_r", (16, 128, D))
    ebr_d = dram_in("eb_row", (1, D))
    pe_d = dram_in("pe_rm", (128, D), F32)
    WL_d = dram_in("WL", (NL, 128, WCOLS))
    BL_d = dram_in("BL", (NL, 128, 20), F32)
    RL_d = dram_in("RL", (NL, 1, 3 * D))
    dga_d = dram_in("dg_attn", (NL, 128, 3 * D))
    dgf_d = dram_in("dg_ffn", (NL, 128, 3 * D))
    idnf_d = dram_in("idn_f", (128, 128), F32)
    idnb_d = dram_in("idn_b", (128, 128))
    onesL_d = dram_in("onesL", (128, 1))
    clsw_d = dram_in("clsw_r", (128, 3), F32)
    eps_d = dram_in("epsc", (128, 1), F32)
    clsb_d = dram_in("clsb", (1, 1), F32)

    y_d = nc.dram_tensor("yc", [RPC, 1], F32, kind="ExternalOutput")

    from contextlib import ExitStack
    with tile.TileContext(nc) as tc, ExitStack() as ctx:
        const = ctx.enter_context(tc.tile_pool(name="const", bufs=1))
        state = ctx.enter_context(tc.tile_pool(name="state", bufs=1))
        psA = ctx.enter_context(tc.tile_pool(name="psA", bufs=2, space="PSUM"))
        psB = ctx.enter_context(tc.tile_pool(name="psB", bufs=2, space="PSUM"))
        psC = ctx.enter_context(tc.tile_pool(name="psC", bufs=2, space="PSUM"))

        # full input in one DMA (CNN-scoped pool, released before transformer)
        def load_const_in(pool, dram, shape, dt):
            nm = dram.name + "_sb"
            t = pool.tile(list(shape), dt, tag=nm, name=nm)
            nc.sync.dma_start(t[:], dram[:])
            return t

        const = ctx.enter_context(tc.tile_pool(name="const", bufs=1))
        state = ctx.enter_context(tc.tile_pool(name="state", bufs=1))
        idn_f = load_const_in(const, idnf_d, (128, 128), F32)
        idn_b = load_const_in(const, idnb_d, (128, 128), BF16)
        onesL = load_const_in(const, onesL_d, (128, 1), BF16)
        clsw = load_const_in(const, clsw_d, (128, 3), F32)
        epsc = load_const_in(const, eps_d, (128, 1), F32)
        clsb = load_const_in(const, clsb_d, (1, 1), F32)
        ones_bf = const.tile([1, 512], BF16, tag="ones_bf", name="ones_bf")
        nc.vector.memset(ones_bf[:], 1.0)

        # persistent state written by CNN: normalized activations, bf16
        xn_rm = state.tile([128, RPC, D], BF16, tag="xn_rm", name="xn_rm")
        xn_fm = state.tile([128, 3, R], BF16, tag="xn_fm", name="xn_fm")

        # transformer weights pool must outlive the CNN block (prefetch L0)
        wpool = ctx.enter_context(tc.tile_pool(name="wpool", bufs=2))

        WLs, BLs, RLs, DGAs, DGFs = [], [], [], [], []
        def load_layer(l):
            wl = wpool.tile([128, WCOLS], BF16, tag="WL", name=f"WL{l}")
            nc.sync.dma_start(wl[:], WL_d[l])
            bl = wpool.tile([128, 20], F32, tag="BL", name=f"BL{l}")
            nc.sync.dma_start(bl[:], BL_d[l])
            rl = wpool.tile([1, 3 * D], BF16, tag="RL", name=f"RL{l}")
            nc.sync.dma_start(rl[:], RL_d[l])
            dga = dgf = None
            if not gp_ident[l]:
                dga = wpool.tile([128, 3 * D], BF16, tag="DGA", name=f"DGA{l}")
                nc.sync.dma_start(dga[:], dga_d[l])
            if not gp_ident[NL + l]:
                dgf = wpool.tile([128, 3 * D], BF16, tag="DGF", name=f"DGF{l}")
                nc.sync.dma_start(dgf[:], dgf_d[l])
            return (wl, bl, rl, dga, dgf)

        cur = load_layer(0)

        # ------------------------------------------------------- CNN + embed
        # 4 row-tiles per group: conv matmuls move 512 cols (4 rts) at once
        with tc.tile_pool(name="cnnc", bufs=1) as cnnc, \
                tc.tile_pool(name="cnn", bufs=2) as cnnp:
            x_all = cnnc.tile([128, RPC, W], F32, tag="x_all", name="x_all")
            nc.sync.dma_start(x_all[:], x_d[:])
            T0v, T1v, T2v = [], [], []
            for conv, (dst, dram, npart) in enumerate(
                    ((T0v, T0_d, 128), (T1v, T1_d, 64), (T2v, T2_d, 64))):
                for vi in range(len(conv_deltas(conv))):
                    t = cnnc.tile([npart, 128], BF16, tag=f"Tv{conv}_{vi}",
                                  name=f"Tv{conv}_{vi}")
                    nc.sync.dma_start(t[:], dram[vi])
                    dst.append(t)
            d2i = [{d: i for i, d in enumerate(conv_deltas(c))} for c in range(3)]
            b0e = load_const_in(cnnc, b0e_d, (128, 1), F32)
            b1e = load_const_in(cnnc, b1e_d, (128, 1), F32)
            b2e = load_const_in(cnnc, b2e_d, (128, 1), F32)
            eb_row = load_const_in(cnnc, ebr_d, (1, D), BF16)
            pe_rm = load_const_in(cnnc, pe_d, (128, D), F32)
            We = []
            for c in range(16):
                t = cnnc.tile([128, D], BF16, tag=f"We{c}", name=f"We{c}")
                nc.sync.dma_start(t[:], We_d[c])
                We.append(t)

            for g in range(2):
                rts = range(g * 4, (g + 1) * 4)
                # transpose x: per rt, both halves -> xt4 [128, half, rt, 128]
                xt4 = cnnp.tile([128, 2, 4, 128], BF16, tag="xt4", name="xt4")
                for j, rt in enumerate(rts):
                    psx = psC.tile([128, 3, 128], F32, tag="psC", name="psC")
                    for half in range(2):
                        nc.tensor.transpose(
                            psx[:, half, :],
                            x_all[:, rt, half * 128:(half + 1) * 128], idn_f[:])
                    if j % 2 == 0:
                        nc.scalar.copy(xt4[:, :, j, :], psx[:, 0:2, :])
                    else:
                        nc.vector.tensor_copy(xt4[:, :, j, :], psx[:, 0:2, :])

                def conv_unit(conv, Tv, srcs, bias, b0, out_cb):
                    """blocks b0, b0+1 x 4 rts -> one 2-bank psum; hi-half
                    relu+bias on ACT -> r_hi; out_cb(ps, r_hi) pools."""
                    ps = psA.tile([128, 2, 512], F32, tag="psA", name="psA")
                    for bi in range(2):
                        ovl = overlaps(conv, b0 + bi)
                        for i, (s, dlt) in enumerate(ovl):
                            nc.tensor.matmul(
                                ps[:, bi, :],
                                lhsT=Tv[d2i[conv][dlt]][:], rhs=srcs(s),
                                start=(i == 0), stop=(i == len(ovl) - 1))
                    r_hi = cnnp.tile([64, 2, 512], BF16, tag="r_hi", name="r_hi")
                    nc.scalar.activation(r_hi[:], ps[64:128], AF.Relu,
                                         bias=bias[64:128, :])
                    out_cb(ps, r_hi)

                # conv0 -> pooled0 [64, 8, 4, 128]
                pooled0 = cnnp.tile([64, NB0, 4, 128], BF16, tag="pooled0",
                                    name="pooled0")
                for b0_ in range(0, NB0, 2):
                    def p0(ps, r_hi, b0_=b0_):
                        nc.vector.scalar_tensor_tensor(
                            pooled0[:, b0_:b0_ + 2, :, :],
                            in0=ps[0:64].rearrange("p a (j r) -> p a j r", j=4),
                            scalar=b0e[0:64, :],
                            in1=r_hi[:].rearrange("p a (j r) -> p a j r", j=4),
                            op0=OP.add, op1=OP.max)
                    conv_unit(0, T0v, lambda s: xt4[:, s, :, :], b0e, b0_, p0)

                # conv1 -> pooled1 [64, 16, 4, 128]
                pooled1 = cnnp.tile([64, NB1, 4, 128], BF16, tag="pooled1",
                                    name="pooled1")
                for b0_ in range(0, NB1, 2):
                    def p1(ps, r_hi, b0_=b0_):
                        nc.vector.scalar_tensor_tensor(
                            pooled1[:, b0_:b0_ + 2, :, :],
                            in0=ps[0:64].rearrange("p a (j r) -> p a j r", j=4),
                            scalar=b1e[0:64, :],
                            in1=r_hi[:].rearrange("p a (j r) -> p a j r", j=4),
                            op0=OP.add, op1=OP.max)
                    conv_unit(1, T1v, lambda s: pooled0[:, s, :, :], b1e, b0_, p1)

                # conv2 -> act3 [128, 16, 4, 128]; parity -> partition half
                act3 = cnnp.tile([128, 16, 4, 128], BF16, tag="act3", name="act3")
                for b0_ in range(0, NB2, 2):
                    def p2(ps, r_hi, b0_=b0_):
                        ch = b0_ // 2
                        nc.vector.scalar_tensor_tensor(
                            act3[0:64, ch, :, :],
                            in0=ps[0:64, 0].rearrange("p (j r) -> p j r", j=4),
                            scalar=b2e[0:64, :],
                            in1=r_hi[:, 0].rearrange("p (j r) -> p j r", j=4),
                            op0=OP.add, op1=OP.max)
                        nc.vector.scalar_tensor_tensor(
                            act3[64:128, ch, :, :],
                            in0=ps[0:64, 1].rearrange("p (j r) -> p j r", j=4),
                            scalar=b2e[0:64, :],
                            in1=r_hi[:, 1].rearrange("p (j r) -> p j r", j=4),
                            op0=OP.add, op1=OP.max)
                    conv_unit(2, T2v, lambda s: pooled1[:, s, :, :], b2e, b0_, p2)

                # embed + bias + relu + pe -> xn_rm / xn_fm per rt
                for j, rt in enumerate(rts):
                    pse = psB.tile([128, 512], F32, tag="psB", name="psB")
                    for c in range(16):
                        nc.tensor.matmul(pse[:, 0:D], lhsT=act3[:, c, j, :],
                                         rhs=We[c][:],
                                         start=(c == 0), stop=False)
                    nc.tensor.matmul(pse[:, 0:D], lhsT=ones_bf[:, 0:128],
                                     rhs=eb_row[:], start=False, stop=True)
                    nc.vector.scalar_tensor_tensor(
                        xn_rm[:, rt, :], in0=pse[:, 0:D], scalar=0.0,
                        in1=pe_rm[:], op0=OP.max, op1=OP.add)
                    psx = psC.tile([128, 3, 128], BF16, tag="psC", name="psC2")
                    for c in range(3):
                        nc.tensor.transpose(psx[:, c, :],
                                            xn_rm[:, rt, c * 128:(c + 1) * 128],
                                            idn_b[:])
                    if rt % 2:
                        nc.vector.tensor_copy(
                            xn_fm[:, :, rt * 128:(rt + 1) * 128], psx[:])
                    else:
                        nc.scalar.copy(xn_fm[:, :, rt * 128:(rt + 1) * 128],
                                       psx[:])

        # transformer-only state (own pool: allocated after CNN pools
        # release so it reuses their SBUF space)
        tstate = ctx.enter_context(tc.tile_pool(name="tstate", bufs=1))
        o_fm = tstate.tile([128, 3, R], BF16, tag="o_fm", name="o_fm")
        h1 = tstate.tile([128, 12, R], BF16, tag="h1", name="h1")
        qc_t = tstate.tile([128, 4, R], BF16, tag="qc_t", name="qc_t")
        kc_t = tstate.tile([128, 4, R], BF16, tag="kc_t", name="kc_t")
        qo_t = tstate.tile([64, 4, R], BF16, tag="qo_t", name="qo_t")
        ko_t = tstate.tile([64, 4, R], BF16, tag="ko_t", name="ko_t")

        # ------------------------------------------------------- transformer
        work = ctx.enter_context(tc.tile_pool(name="work", bufs=3))
        lnw = ctx.enter_context(tc.tile_pool(name="lnw", bufs=2))

        def ln_half(rts, px_of):
            """Half-batch layernorm: for rts, px_of(rt) emits matmuls into a
            fresh psB and returns it (pre-LN sum incl. residual+bias).
            Writes xn_rm / xn_fm."""
            x1s = {}
            st = lnw.tile([128, 8, 2], F32, tag="st", name="st")  # s1, s2
            for j, rt in enumerate(rts):
                px = px_of(rt)
                x1 = lnw.tile([128, D], BF16, tag=f"x1_{j}", name=f"x1_{j}")
                # copy + running sum  (rotate ACT / Pool)
                if j % 2 == 0:
                    nc.scalar.activation(x1[:], px[:, 0:D], AF.Identity,
                                         accum_out=st[:, j, 0:1])
                else:
                    nc.vector.tensor_scalar(x1[:], px[:, 0:D], 0.0, 0.0, OP.add,
                                            OP.add, accum_out=st[:, j, 0:1])
                xsq = work.tile([128, D], BF16, tag="xsq", name="xsq")
                nc.vector.scalar_tensor_tensor(
                    xsq[:], in0=x1[:], scalar=0.0, in1=x1[:],
                    op0=OP.add, op1=OP.mult, accum_out=st[:, j, 1:2])
                x1s[rt] = x1
            # mean = s1/D; var = s2/D - mean^2 ; rstd = 1/sqrt(var+eps)
            nb = len(rts)
            mean = lnw.tile([128, 8], F32, tag="mean", name="mean")
            var = lnw.tile([128, 8], F32, tag="var", name="var")
            nc.vector.tensor_scalar(mean[:, 0:nb], st[:, 0:nb, 0], 1.0 / D, None,
                                    OP.mult)
            msq = work.tile([128, 8], F32, tag="msq", name="msq")
            nc.vector.tensor_tensor(msq[:, 0:nb], mean[:, 0:nb], mean[:, 0:nb],
                                    OP.mult)
            nc.vector.scalar_tensor_tensor(
                var[:, 0:nb], in0=st[:, 0:nb, 1], scalar=1.0 / D, in1=msq[:, 0:nb],
                op0=OP.mult, op1=OP.subtract)
            sd = work.tile([128, 8], F32, tag="sd", name="sd")
            nc.scalar.activation(sd[:, 0:nb], var[:, 0:nb], AF.Sqrt, bias=epsc[:])
            rstd = lnw.tile([128, 8], F32, tag="rstd", name="rstd")
            nc.vector.reciprocal(rstd[:, 0:nb], sd[:, 0:nb])
            mrs = lnw.tile([128, 8], F32, tag="mrs", name="mrs")
            nc.vector.scalar_tensor_tensor(
                mrs[:, 0:nb], in0=mean[:, 0:nb], scalar=-1.0, in1=rstd[:, 0:nb],
                op0=OP.mult, op1=OP.mult)
            for j, rt in enumerate(rts):
                nc.vector.tensor_scalar(
                    xn_rm[:, rt, :], x1s[rt][:], rstd[:, j:j + 1],
                    mrs[:, j:j + 1], OP.mult, OP.add)
                pst = psC.tile([128, 3, 128], BF16, tag="psC", name="psC")
                for c in range(3):
                    nc.tensor.transpose(pst[:, c, :],
                                        xn_rm[:, rt, c * 128:(c + 1) * 128],
                                        idn_b[:])
                if j % 2 == 0:
                    nc.vector.tensor_copy(
                        xn_fm[:, :, rt * 128:(rt + 1) * 128], pst[:])
                else:
                    nc.scalar.copy(xn_fm[:, :, rt * 128:(rt + 1) * 128], pst[:])

        for lyr in range(n_layers):
            wl, bl, rl, dga, dgf = cur
            if lyr + 1 < n_layers:
                cur = load_layer(lyr + 1)

            def wv_(c):   # [128, 384] views into the blob
                return wl[:, c * CSEC + 1024:c * CSEC + 1408]
            def wo_(c):
                return wl[:, c * CSEC + 1408:c * CSEC + 1792]
            def w1_(c):
                return wl[:, c * CSEC + 1792:c * CSEC + 3328]
            def w2_(dc):
                return wl[:, 3 * CSEC + dc * 384:3 * CSEC + (dc + 1) * 384]

            if phase < 2:
                continue
            # batched Q/K into head-padded feature-major tiles (4 out-chunks)
            for pi, (dstc, woff, bcol) in enumerate(
                    ((qc_t, 0, 0), (kc_t, 512, 4))):
                for oc in range(4):
                    pq = psA.tile([128, 2, 512], F32, tag="psA", name="psA")
                    for hf in range(2):
                        for c in range(3):
                            nc.tensor.matmul(
                                pq[:, hf, :],
                                lhsT=wl[:, c * CSEC + woff + oc * 128:
                                        c * CSEC + woff + (oc + 1) * 128],
                                rhs=xn_fm[:, c, hf * 512:(hf + 1) * 512],
                                start=(c == 0), stop=(c == 2))
                    eng = (nc.scalar, nc.vector)[(pi * 4 + oc) % 2]
                    if eng is nc.scalar:
                        nc.scalar.activation(dstc[:, oc, :],
                                             pq[:].rearrange("p a b -> p (a b)"),
                                             AF.Identity,
                                             bias=bl[:, bcol + oc:bcol + oc + 1])
                    else:
                        eng.tensor_scalar(dstc[:, oc, :],
                                          pq[:].rearrange("p a b -> p (a b)"),
                                          bl[:, bcol + oc:bcol + oc + 1], None,
                                          OP.add)
            nc.sync.dma_start(qo_t[:], qc_t[64:128, :, :])
            nc.sync.dma_start(ko_t[:], kc_t[64:128, :, :])

            # per-sample attention
            for n in range(RPC):
                cs = slice(n * 128, (n + 1) * 128)
                if phase < 3:
                    break
                # V (+ bias fold) -> v_ext with ones column per head
                pv = psB.tile([128, 512], F32, tag="psB", name="psB")
                for c in range(3):
                    nc.tensor.matmul(pv[:, 0:D], lhsT=xn_fm[:, c, cs], rhs=wv_(c),
                                     start=(c == 0), stop=False)
                nc.tensor.matmul(pv[:, 0:D], lhsT=ones_bf[:, 0:128],
                                 rhs=rl[:, 0:D], start=False, stop=True)
                v_ext = work.tile([128, H, E + 1], BF16, tag="v_ext", name="v_ext")
                nc.vector.memset(v_ext[:, :, E:E + 1], 1.0)
                nc.vector.tensor_copy(
                    v_ext[:, :, 0:E],
                    pv[:, 0:D].rearrange("p (h e) -> p h e", h=H))
                if phase < 4:
                    continue

                # scores S^T = K^T Q per head (all operands base partition 0)
                pss = psA.tile([128, 2, 512], F32, tag="psA", name="psA")
                for h in range(H):
                    kk = kc_t if h % 2 == 0 else ko_t
                    qq = qc_t if h % 2 == 0 else qo_t
                    nc.tensor.matmul(
                        pss[:, h // 4, (h % 4) * 128:(h % 4 + 1) * 128],
                        lhsT=kk[0:E, h // 2, cs],
                        rhs=qq[0:E, h // 2, cs],
                        start=True, stop=True)
                es16 = work.tile([128, H, 128], BF16, tag="es16", name="es16")
                if split_exp:
                    for hf in range(2):
                        nc.scalar.activation(
                            es16[:, hf * 4:(hf + 1) * 4, :],
                            pss[:, hf].rearrange("p (b r) -> p b r", b=4),
                            AF.Exp, scale=TEMP)
                else:
                    nc.scalar.activation(
                        es16[:], pss[:].rearrange("p a (b r) -> p (a b) r", b=4),
                        AF.Exp, scale=TEMP)
                if phase < 5:
                    continue

                # A·[V|1] -> per-head 49-col groups: o unnormalized + row sums
                pso = psB.tile([128, 512], F32, tag="psB", name="psO")
                for h in range(H):
                    nc.tensor.matmul(pso[:, h * 49:(h + 1) * 49],
                                     lhsT=es16[:, h, :], rhs=v_ext[:, h, :],
                                     start=True, stop=True)
                pso_v = pso[:, 0:392].rearrange("p (h e) -> p h e", h=H)
                rr = work.tile([128, H], F32, tag="rr", name="rr")
                nc.vector.reciprocal(rr[:], pso_v[:, :, E])
                o_rm = work.tile([128, D], BF16, tag="o_rm", name="o_rm")
                rrb = AP(rr.tensor, rr.offset, [list(rr.ap[0]), [1, H], [0, E]])
                nc.vector.tensor_tensor(o_rm[:].rearrange("p (h e) -> p h e", h=H),
                                        pso_v[:, :, 0:E], rrb, OP.mult)
                ps = psC.tile([128, 3, 128], BF16, tag="psC", name="psC")
                for c in range(3):
                    nc.tensor.transpose(ps[:, c, :], o_rm[:, c * 128:(c + 1) * 128],
                                        idn_b[:])
                if n % 2:
                    nc.vector.tensor_copy(o_fm[:, :, cs], ps[:])
                else:
                    nc.scalar.copy(o_fm[:, :, cs], ps[:])

            if phase < 6:
                continue

            # LN1: px = o@Wo + bo' + resid(xn)
            def attn_px(rt):
                cs = slice(rt * 128, (rt + 1) * 128)
                px = psB.tile([128, 512], F32, tag="psB", name="psB")
                for c in range(3):
                    nc.tensor.matmul(px[:, 0:D], lhsT=o_fm[:, c, cs], rhs=wo_(c),
                                     start=(c == 0), stop=False)
                nc.tensor.matmul(px[:, 0:D], lhsT=ones_bf[:, 0:128],
                                 rhs=rl[:, D:2 * D], start=False, stop=False)
                if dga is None:
                    nc.tensor.matmul(px[:, 0:D], lhsT=idn_b[:], rhs=xn_rm[:, rt, :],
                                     start=False, stop=True)
                else:
                    dv = dga[:].rearrange("p (c d) -> p c d", c=3)
                    for c in range(3):
                        nc.tensor.matmul(px[:, 0:D], lhsT=xn_fm[:, c, cs],
                                         rhs=dv[:, c], start=False, stop=(c == 2))
                return px

            ln_half(range(0, 4), attn_px)
            ln_half(range(4, 8), attn_px)

            if phase < 7:
                continue
            # FFN: h1 = relu(xn @ W1' + b1')
            for dc in range(12):
                ph = psA.tile([128, 2, 512], F32, tag="psA", name="psA")
                for hf in range(2):
                    for c in range(3):
                        nc.tensor.matmul(
                            ph[:, hf, :],
                            lhsT=w1_(c)[:, dc * 128:(dc + 1) * 128],
                            rhs=xn_fm[:, c, hf * 512:(hf + 1) * 512],
                            start=(c == 0), stop=(c == 2))
                eng = (nc.scalar, nc.vector)[dc % 2]
                if eng is nc.scalar:
                    nc.scalar.activation(h1[:, dc, :],
                                         ph[:].rearrange("p a b -> p (a b)"),
                                         AF.Relu, bias=bl[:, 8 + dc:9 + dc])
                else:
                    eng.tensor_scalar(h1[:, dc, :],
                                      ph[:].rearrange("p a b -> p (a b)"),
                                      bl[:, 8 + dc:9 + dc], 0.0, OP.add, OP.max)
            if phase < 8:
                continue

            # LN2: px = h1@W2 + b2' + resid(xn)
            def ffn_px(rt):
                cs = slice(rt * 128, (rt + 1) * 128)
                px = psB.tile([128, 512], F32, tag="psB", name="psB")
                for dc in range(12):
                    nc.tensor.matmul(px[:, 0:D], lhsT=h1[:, dc, cs], rhs=w2_(dc),
                                     start=(dc == 0), stop=False)
                nc.tensor.matmul(px[:, 0:D], lhsT=ones_bf[:, 0:128],
                                 rhs=rl[:, 2 * D:3 * D], start=False, stop=False)
                if dgf is None:
                    nc.tensor.matmul(px[:, 0:D], lhsT=idn_b[:], rhs=xn_rm[:, rt, :],
                                     start=False, stop=True)
                else:
                    dv = dgf[:].rearrange("p (c d) -> p c d", c=3)
                    for c in range(3):
                        nc.tensor.matmul(px[:, 0:D], lhsT=xn_fm[:, c, cs],
                                         rhs=dv[:, c], start=False, stop=(c == 2))
                return px

            ln_half(range(0, 4), ffn_px)
            ln_half(range(4, 8), ffn_px)

        # ------------------------------------------------------- head
        outsb = state.tile([1, RPC], F32, tag="outsb", name="outsb")
        for n in range(RPC):
            pm = psC.tile([128, 3], F32, tag="psC", name="psCh")
            for c in range(3):
                nc.tensor.matmul(pm[:, c:c + 1],
                                 lhsT=xn_rm[:, n, c * 128:(c + 1) * 128],
                                 rhs=onesL[:], start=True, stop=True)
            tm = work.tile([128, 3], F32, tag="tm", name="tm")
            nc.scalar.copy(tm[:], pm[:])
            pc = psC.tile([1, 8], F32, tag="psC", name="psCh2")
            for c in range(3):
                nc.tensor.matmul(pc[:, 0:1], lhsT=tm[:, c:c + 1],
                                 rhs=clsw[:, c:c + 1],
                                 start=(c == 0), stop=(c == 2))
            nc.scalar.activation(outsb[:, n:n + 1], pc[:, 0:1], AF.Identity,
                                 bias=clsb[:])
        nc.sync.dma_start(y_d[:].rearrange("a b -> b a"), outsb[:])

    if do_compile:
        nc.compile()
    return nc


_PROG = {}


def _get_prog(gp_ident=None, n_layers=NL, phase=99):
    key = (tuple(gp_ident) if gp_ident else None, n_layers, phase)
    if key not in _PROG:
        _PROG[key] = build_program(gp_ident, n_layers=n_layers, phase=phase)
    return _PROG[key]


def _in_maps(inputs):
    shared = host_prep(inputs)
    gp_ident = shared.pop("gp_identity")
    x = np.asarray(inputs["x"], np.float32)  # (64, 128, 256)
    in_maps = []
    for c in range(NCORES):
        m = dict(shared)
        # [128 tokens, rt, W] per core
        xc = x[c * RPC:(c + 1) * RPC]              # (8, 128, 256)
        m["xc"] = np.ascontiguousarray(xc.transpose(1, 0, 2))
        in_maps.append(m)
    return in_maps, gp_ident


def kernel(**inputs):
    in_maps, gp_ident = _in_maps(inputs)
    nc = _get_prog(gp_ident)
    res = run_bass_kernel_spmd(nc, in_maps, core_ids=list(range(NCORES)))
    out = np.concatenate([res.results[c]["yc"] for c in range(NCORES)], axis=0)
    return out.astype(np.float32)


def timed_run(inputs, iters=32):
    """Estimate per-execution HW time by chaining NEFF executions.

    No NTFF hook is available through this axon tunnel, so true HW exec
    time can't be read from a profile.  Instead we chain k executions
    (each iteration's outputs feed the next call's operands, forcing
    device-side serialization while dispatch pipelines) and report the
    marginal wall time per added execution:  (t_k - t_1) / (k - 1).
    This subtracts the fixed per-dispatch tunnel overhead (~80 ms) that
    would otherwise swamp the measurement.  Returns ns.
    """
    import time
    import jax
    from jax.experimental.shard_map import shard_map
    from jax.sharding import Mesh, NamedSharding, PartitionSpec
    from concourse import bass2jax, mybir as mb

    in_maps, gp_ident = _in_maps(inputs)
    nc = _get_prog(gp_ident)
    bass2jax.install_neuronx_cc_hook()
    partition_name = nc.partition_id_tensor.name if nc.partition_id_tensor else None
    in_names, out_names, out_avals, zero_outs = [], [], [], []
    for alloc in nc.m.functions[0].allocations:
        if not isinstance(alloc, mb.MemoryLocationSet):
            continue
        name = alloc.memorylocations[0].name
        if alloc.kind == "ExternalInput":
            if name != partition_name:
                in_names.append(name)
        elif alloc.kind == "ExternalOutput":
            shape = tuple(alloc.tensor_shape)
            dtype = mb.dt.np(alloc.dtype)
            out_avals.append(jax.core.ShapedArray(shape, dtype))
            out_names.append(name)
            zero_outs.append(np.zeros(shape, dtype))
    n_params, n_outs = len(in_names), len(out_avals)
    all_in = list(in_names) + list(out_names)
    if partition_name is not None:
        all_in.append(partition_name)

    def _body(*args):
        ins = list(args[:n_params])
        outs = list(args[n_params:])
        operands = ins + outs
        if partition_name is not None:
            operands = operands + [bass2jax.partition_id_tensor()]
        outs = list(bass2jax._bass_exec_p.bind(
            *operands, out_avals=tuple(out_avals), in_names=tuple(all_in),
            out_names=tuple(out_names), lowering_input_output_aliases=(),
            sim_require_finite=True, sim_require_nnan=True, nc=nc))
        return tuple(outs)

    devices = jax.devices()[:NCORES]
    mesh = Mesh(np.asarray(devices), ("core",))
    shard = NamedSharding(mesh, PartitionSpec("core"))
    dev_in = [jax.device_put(
        np.concatenate([np.asarray(in_maps[c][nm]) for c in range(NCORES)], axis=0),
        shard) for nm in in_names]
    zsh = [np.zeros((NCORES * z.shape[0], *z.shape[1:]), z.dtype) for z in zero_outs]

    f = jax.jit(
        shard_map(_body, mesh=mesh,
                  in_specs=(PartitionSpec("core"),) * (n_params + n_outs),
                  out_specs=(PartitionSpec("core"),) * n_outs, check_rep=False),
        keep_unused=True)

    def run_chain(k):
        outs = [jax.device_put(z, shard) for z in zsh]
        jax.block_until_ready(outs)
        jax.block_until_ready(dev_in)
        t0 = time.perf_counter()
        for _ in range(k):
            outs = list(f(*dev_in, *outs))
        jax.block_until_ready(outs)
        return time.perf_counter() - t0

    run_chain(1)  # warm compile
    t1 = min(run_chain(1) for _ in range(3))
    tk = min(run_chain(iters) for _ in range(3))
    return int((tk - t1) / (iters - 1) * 1e9)




# revision 3
# speedup vs baseline: 1.1660x; 1.1660x over previous
"""Trainium2 Bass kernel v3 for nn_ClassificationModel.

Data parallel across 8 NeuronCores: batch N=64 -> 8 samples/core.

Differences vs v2 baseline:
  - All per-layer transformer weights packed into ONE [128, 13824] bf16 DMA
    (one HWDGE acquire per layer instead of ~40), double-buffered prefetch.
  - Scores read compact feature-major Q/K via partition-offset lhsT slices
    (heads 2/5 split across chunk boundaries -> 2 accumulating matmuls);
    the per-head 64-row spread DMAs are gone entirely.
  - LayerNorm affine (g, be) folded into adjacent weights host-side; device
    state is the *normalized* activation xn in bf16 (row- and feature-major).
    Residual enters the pre-LN sum as a single identity matmul.  LN stats
    come from accum_out sums (copy + square), apply is one tensor_scalar.
  - 2-PSUM-bank (free=1024) elementwise units for QK bias, scores exp and
    FFN relu; PSUM-consuming ops rotated across ACT/DVE/Pool for balance.
"""

import math
import sys

sys.path.insert(0, "/opt/trn_rl_repo")

import numpy as np
import ml_dtypes

import concourse.bass as bass
import concourse.mybir as mybir
import concourse.tile as tile
from concourse import bacc
from concourse.bass import AP
from concourse.bass_utils import run_bass_kernel_spmd

BF = ml_dtypes.bfloat16
F32 = mybir.dt.float32
BF16 = mybir.dt.bfloat16
AX = mybir.AxisListType
OP = mybir.AluOpType
AF = mybir.ActivationFunctionType

# model dims
N, L, W = 64, 128, 256
D, H, NL, DFF = 384, 8, 4, 1536
E = D // H  # 48
CH = [1, 4, 16, 64]
K = 7
NCORES = 8
RPC = N // NCORES          # samples per core = 8
R = RPC * L                # rows per core = 1024
TEMP = 1.0 / math.sqrt(E)
EPS = 1e-5

# conv block sizes (output positions per Toeplitz block)
B0, B1, B2 = 32, 8, 2
NB0, NB1, NB2 = 256 // B0, 128 // B1, 64 // B2  # 8, 16, 32

# packed per-layer weight blob column offsets (bf16, [128, WCOLS])
# chunk c in 0..2: Wq(512, head-padded) Wk(512) Wv(384) Wo(384) W1(1536)
# at c*3328; then W2: 12 chunks x 384 at 9984.
# Q/K output features are head-padded: head h -> rows 64h..64h+47 of 512,
# so per-head score matmuls read base partitions 0/64 (hw constraint).
CSEC = 512 + 512 + 384 + 384 + 1536  # 3328
WCOLS = 3 * CSEC + 12 * 384          # 14592


# ---------------------------------------------------------------------------
# host-side weight preparation
# ---------------------------------------------------------------------------

def _pe_np(l, d):
    pos = np.arange(l)[:, None].astype(np.float32)
    i = np.arange(d // 2)[None, :].astype(np.float32)
    ang = pos / np.power(10000.0, 2.0 * i / d)
    pe = np.zeros((l, d), np.float32)
    pe[:, 0::2] = np.sin(ang)
    pe[:, 1::2] = np.cos(ang)
    return pe


# conv source-block overlap enumeration (shared host/device) -----------------

CONV_GEOM = {
    0: (B0, 128, 2, 1),
    1: (B1, 16, NB0, 4),
    2: (B2, 4, NB1, 16),
}


def overlaps(conv, b):
    Bout, src_size, nsrc, _ = CONV_GEOM[conv]
    w0, w1 = Bout * b - 3, Bout * b + Bout + 3
    res = []
    for s in range(nsrc):
        lo, hi = s * src_size, (s + 1) * src_size
        if max(w0, lo) < min(w1, hi):
            res.append((s, lo - Bout * b))
    return res


def conv_deltas(conv):
    nb = {0: NB0, 1: NB1, 2: NB2}[conv]
    return sorted({d for b in range(nb) for _, d in overlaps(conv, b)})


def _m_layout(conv, h, co):
    if conv == 0:
        return (h & 1) * 64 + (h >> 1) * 4 + co
    if conv == 1:
        return (h & 1) * 64 + (h >> 1) * 16 + co
    return h * 64 + co


def _toeplitz_variants(conv, w):
    Bout, src_size, _, nch = CONV_GEOM[conv]
    cout = w.shape[0]
    ds = conv_deltas(conv)
    T = np.zeros((len(ds), src_size * nch, 128), np.float32)
    for vi, delta in enumerate(ds):
        for hp in range(src_size):
            for h in range(Bout):
                k = delta + hp - h + 3
                if 0 <= k < K:
                    for co in range(cout):
                        for ci in range(nch):
                            T[vi, hp * nch + ci, _m_layout(conv, h, co)] = w[co, ci, k]
    return T


def host_prep(inp):
    d = {}
    f32 = np.float32
    d["T0"] = _toeplitz_variants(0, np.asarray(inp["conv_w0"], f32)).astype(BF)
    d["T1"] = _toeplitz_variants(1, np.asarray(inp["conv_w1"], f32)).astype(BF)
    d["T2"] = _toeplitz_variants(2, np.asarray(inp["conv_w2"], f32)).astype(BF)
    b0, b1c, b2c = (np.asarray(inp[f"conv_b{i}"], f32) for i in range(3))
    p = np.arange(128)
    d["b0e"] = b0[p % 4].reshape(128, 1)
    d["b1e"] = b1c[p % 16].reshape(128, 1)
    d["b2e"] = b2c[p % 64].reshape(128, 1)

    # embed: We_r[c, p, :] = embed_w[(p%64)*32 + 2c + p//64, :]
    ew = np.asarray(inp["embed_w"], f32)  # (2048, 384)
    We_r = np.zeros((16, 128, D), f32)
    for c in range(16):
        for pi in range(128):
            We_r[c, pi] = ew[(pi % 64) * 32 + 2 * c + pi // 64]
    d["We_r"] = We_r.astype(BF)
    d["eb_row"] = np.asarray(inp["embed_b"], f32).reshape(1, D).astype(BF)
    d["pe_rm"] = _pe_np(L, D)

    g1 = np.asarray(inp["g1"], f32)
    be1 = np.asarray(inp["be1"], f32)
    g2 = np.asarray(inp["g2"], f32)
    be2 = np.asarray(inp["be2"], f32)

    # pending affine entering each layer's attention block
    gp = np.stack([np.ones(D, f32) if l == 0 else g2[l - 1] for l in range(NL)])
    bp = np.stack([np.zeros(D, f32) if l == 0 else be2[l - 1] for l in range(NL)])
    d["gp_identity"] = [bool(np.all(gp[l] == 1.0)) for l in range(NL)] + \
                       [bool(np.all(g1[l] == 1.0)) for l in range(NL)]

    Wq = np.asarray(inp["Wq"], f32)
    Wk = np.asarray(inp["Wk"], f32)
    Wv = np.asarray(inp["Wv"], f32)
    Wo = np.asarray(inp["Wo"], f32)
    W1 = np.asarray(inp["W1"], f32)
    W2 = np.asarray(inp["W2"], f32)
    bq = np.asarray(inp["bq"], f32)
    bk = np.asarray(inp["bk"], f32)
    bv = np.asarray(inp["bv"], f32)
    bo = np.asarray(inp["bo"], f32)
    b1 = np.asarray(inp["b1"], f32)
    b2 = np.asarray(inp["b2"], f32)

    # fold pending affines into weights/biases
    WqF = gp[:, :, None] * Wq
    WkF = gp[:, :, None] * Wk
    WvF = gp[:, :, None] * Wv
    bqF = bq + np.einsum("ld,lde->le", bp, Wq)
    bkF = bk + np.einsum("ld,lde->le", bp, Wk)
    bvF = bv + np.einsum("ld,lde->le", bp, Wv)
    W1F = g1[:, :, None] * W1
    b1F = b1 + np.einsum("ld,lde->le", be1, W1)
    boF = bo + bp          # LN1 pre-sum bias includes pending be
    b2F = b2 + be1         # LN2 pre-sum bias includes LN1's be

    # diag blocks for non-identity pending g (3 chunks of [128, 384] each)
    def diag_chunks(g):
        out = np.zeros((128, 3 * D), f32)
        for c in range(3):
            for i in range(128):
                out[i, c * D + c * 128 + i] = g[c * 128 + i]
        return out
    d["dg_attn"] = np.stack([diag_chunks(gp[l]) for l in range(NL)]).astype(BF)
    d["dg_ffn"] = np.stack([diag_chunks(g1[l]) for l in range(NL)]).astype(BF)

    # head-pad Q/K output features: head h -> cols 64h..64h+47 of 512
    def head_pad_w(w):  # (NL, 384, 384) -> (NL, 384, 512)
        out = np.zeros((NL, D, 512), f32)
        for h in range(H):
            out[:, :, 64 * h:64 * h + E] = w[:, :, E * h:E * (h + 1)]
        return out

    def head_pad_b(b):  # (NL, 384) -> (NL, 512)
        out = np.zeros((NL, 512), f32)
        for h in range(H):
            out[:, 64 * h:64 * h + E] = b[:, E * h:E * (h + 1)]
        return out

    WqP, WkP = head_pad_w(WqF), head_pad_w(WkF)
    bqP, bkP = head_pad_b(bqF), head_pad_b(bkF)

    # mega weight blob per layer
    WL = np.zeros((NL, 128, WCOLS), f32)
    for l in range(NL):
        for c in range(3):
            r = slice(c * 128, (c + 1) * 128)
            base = c * CSEC
            WL[l, :, base + 0:base + 512] = WqP[l][r]
            WL[l, :, base + 512:base + 1024] = WkP[l][r]
            WL[l, :, base + 1024:base + 1408] = WvF[l][r]
            WL[l, :, base + 1408:base + 1792] = Wo[l][r]
            WL[l, :, base + 1792:base + 3328] = W1F[l][r]
        for dc in range(12):
            WL[l, :, 3 * CSEC + dc * 384:3 * CSEC + (dc + 1) * 384] = \
                W2[l][dc * 128:(dc + 1) * 128]
    d["WL"] = WL.astype(BF)

    # per-layer f32 bias blob [128, 20]: bq (4 padded chunks), bk, b1r (12)
    BL = np.zeros((NL, 128, 20), f32)
    for oc in range(4):
        BL[:, :, oc] = bqP[:, oc * 128:(oc + 1) * 128]
        BL[:, :, 4 + oc] = bkP[:, oc * 128:(oc + 1) * 128]
    for l in range(NL):
        BL[l, :, 8:20] = b1F[l].reshape(12, 128).T
    d["BL"] = BL

    # per-layer bf16 rows blob [1, 1152]: bv | bo' | b2'
    RL = np.zeros((NL, 1, 3 * D), f32)
    RL[:, 0, 0:D] = bvF
    RL[:, 0, D:2 * D] = boF
    RL[:, 0, 2 * D:3 * D] = b2F
    d["RL"] = RL.astype(BF)

    d["idn_f"] = np.eye(128, dtype=f32)
    d["idn_b"] = np.eye(128, dtype=f32).astype(BF)
    d["onesL"] = np.full((128, 1), 1.0 / L, f32).astype(BF)
    # head fold: mean(g2[3]*xn + be2[3]) @ cls_w + cls_b
    cw = np.asarray(inp["cls_w"], f32)          # (384, 1)
    cb = np.asarray(inp["cls_b"], f32)          # (1,)
    cwF = (g2[NL - 1][:, None] * cw)
    cbF = cb + be2[NL - 1] @ cw
    d["clsw_r"] = cwF.reshape(3, 128).T.copy()  # (128, 3)
    d["clsb"] = cbF.reshape(1, 1)
    d["epsc"] = np.full((128, 1), EPS, f32)
    return d


# ---------------------------------------------------------------------------
# device program
# ---------------------------------------------------------------------------

def build_program(gp_ident=None, do_compile=True, n_layers=NL, phase=99, split_exp=False, pad_scores=False, even_only=False):
    if gp_ident is None:
        gp_ident = [True] * (2 * NL)
    nc = bacc.Bacc("TRN2", target_bir_lowering=False, debug=False)

    def dram_in(name, shape, dt=BF16):
        return nc.dram_tensor(name, list(shape), dt, kind="ExternalInput")

    x_d = dram_in("xc", (128, RPC, W), F32)
    nv0, nv1, nv2 = len(conv_deltas(0)), len(conv_deltas(1)), len(conv_deltas(2))
    T0_d = dram_in("T0", (nv0, 128, 128))
    T1_d = dram_in("T1", (nv1, 64, 128))
    T2_d = dram_in("T2", (nv2, 64, 128))
    b0e_d = dram_in("b0e", (128, 1), F32)
    b1e_d = dram_in("b1e", (128, 1), F32)
    b2e_d = dram_in("b2e", (128, 1), F32)
    We_d = dram_in("We_r", (16, 128, D))
    ebr_d = dram_in("eb_row", (1, D))
    pe_d = dram_in("pe_rm", (128, D), F32)
    WL_d = dram_in("WL", (NL, 128, WCOLS))
    BL_d = dram_in("BL", (NL, 128, 20), F32)
    RL_d = dram_in("RL", (NL, 1, 3 * D))
    dga_d = dram_in("dg_attn", (NL, 128, 3 * D))
    dgf_d = dram_in("dg_ffn", (NL, 128, 3 * D))
    idnf_d = dram_in("idn_f", (128, 128), F32)
    idnb_d = dram_in("idn_b", (128, 128))
    onesL_d = dram_in("onesL", (128, 1))
    clsw_d = dram_in("clsw_r", (128, 3), F32)
    eps_d = dram_in("epsc", (128, 1), F32)
    clsb_d = dram_in("clsb", (1, 1), F32)

    y_d = nc.dram_tensor("yc", [RPC, 1], F32, kind="ExternalOutput")

    from contextlib import ExitStack
    with tile.TileContext(nc) as tc, ExitStack() as ctx:
        const = ctx.enter_context(tc.tile_pool(name="const", bufs=1))
        state = ctx.enter_context(tc.tile_pool(name="state", bufs=1))
        psA = ctx.enter_context(tc.tile_pool(name="psA", bufs=2, space="PSUM"))
        psB = ctx.enter_context(tc.tile_pool(name="psB", bufs=2, space="PSUM"))
        psC = ctx.enter_context(tc.tile_pool(name="psC", bufs=2, space="PSUM"))

        # full input in one DMA (CNN-scoped pool, released before transformer)
        def load_const_in(pool, dram, shape, dt):
            nm = dram.name + "_sb"
            t = pool.tile(list(shape), dt, tag=nm, name=nm)
            nc.sync.dma_start(t[:], dram[:])
            return t

        const = ctx.enter_context(tc.tile_pool(name="const", bufs=1))
        state = ctx.enter_context(tc.tile_pool(name="state", bufs=1))
        idn_f = load_const_in(const, idnf_d, (128, 128), F32)
        idn_b = load_const_in(const, idnb_d, (128, 128), BF16)
        onesL = load_const_in(const, onesL_d, (128, 1), BF16)
        clsw = load_const_in(const, clsw_d, (128, 3), F32)
        epsc = load_const_in(const, eps_d, (128, 1), F32)
        clsb = load_const_in(const, clsb_d, (1, 1), F32)
        ones_bf = const.tile([1, 512], BF16, tag="ones_bf", name="ones_bf")
        nc.vector.memset(ones_bf[:], 1.0)

        # persistent state written by CNN: normalized activations, bf16
        xn_rm = state.tile([128, RPC, D], BF16, tag="xn_rm", name="xn_rm")
        xn_fm = state.tile([128, 3, R], BF16, tag="xn_fm", name="xn_fm")

        # transformer weights pool must outlive the CNN block (prefetch L0)
        wpool = ctx.enter_context(tc.tile_pool(name="wpool", bufs=2))

        WLs, BLs, RLs, DGAs, DGFs = [], [], [], [], []
        def load_layer(l):
            wl = wpool.tile([128, WCOLS], BF16, tag="WL", name=f"WL{l}")
            nc.sync.dma_start(wl[:], WL_d[l])
            bl = wpool.tile([128, 20], F32, tag="BL", name=f"BL{l}")
            nc.sync.dma_start(bl[:], BL_d[l])
            rl = wpool.tile([1, 3 * D], BF16, tag="RL", name=f"RL{l}")
            nc.sync.dma_start(rl[:], RL_d[l])
            dga = dgf = None
            if not gp_ident[l]:
                dga = wpool.tile([128, 3 * D], BF16, tag="DGA", name=f"DGA{l}")
                nc.sync.dma_start(dga[:], dga_d[l])
            if not gp_ident[NL + l]:
                dgf = wpool.tile([128, 3 * D], BF16, tag="DGF", name=f"DGF{l}")
                nc.sync.dma_start(dgf[:], dgf_d[l])
            return (wl, bl, rl, dga, dgf)

        cur = load_layer(0)

        # ------------------------------------------------------- CNN + embed
        # 4 row-tiles per group: conv matmuls move 512 cols (4 rts) at once
        with tc.tile_pool(name="cnnc", bufs=1) as cnnc, \
                tc.tile_pool(name="cnn", bufs=2) as cnnp:
            x_all = cnnc.tile([128, RPC, W], F32, tag="x_all", name="x_all")
            nc.sync.dma_start(x_all[:], x_d[:])
            T0v, T1v, T2v = [], [], []
            for conv, (dst, dram, npart) in enumerate(
                    ((T0v, T0_d, 128), (T1v, T1_d, 64), (T2v, T2_d, 64))):
                for vi in range(len(conv_deltas(conv))):
                    t = cnnc.tile([npart, 128], BF16, tag=f"Tv{conv}_{vi}",
                                  name=f"Tv{conv}_{vi}")
                    nc.sync.dma_start(t[:], dram[vi])
                    dst.append(t)
            d2i = [{d: i for i, d in enumerate(conv_deltas(c))} for c in range(3)]
            b0e = load_const_in(cnnc, b0e_d, (128, 1), F32)
            b1e = load_const_in(cnnc, b1e_d, (128, 1), F32)
            b2e = load_const_in(cnnc, b2e_d, (128, 1), F32)
            eb_row = load_const_in(cnnc, ebr_d, (1, D), BF16)
            pe_rm = load_const_in(cnnc, pe_d, (128, D), F32)
            We = []
            for c in range(16):
                t = cnnc.tile([128, D], BF16, tag=f"We{c}", name=f"We{c}")
                nc.sync.dma_start(t[:], We_d[c])
                We.append(t)

            for g in range(2):
                rts = range(g * 4, (g + 1) * 4)
                # transpose x: per rt, both halves -> xt4 [128, half, rt, 128]
                xt4 = cnnp.tile([128, 2, 4, 128], BF16, tag="xt4", name="xt4")
                for j, rt in enumerate(rts):
                    psx = psC.tile([128, 3, 128], F32, tag="psC", name="psC")
                    for half in range(2):
                        nc.tensor.transpose(
                            psx[:, half, :],
                            x_all[:, rt, half * 128:(half + 1) * 128], idn_f[:])
                    if j % 2 == 0:
                        nc.scalar.copy(xt4[:, :, j, :], psx[:, 0:2, :])
                    else:
                        nc.vector.tensor_copy(xt4[:, :, j, :], psx[:, 0:2, :])

                def conv_unit(conv, Tv, srcs, bias, b0, out_cb):
                    """blocks b0, b0+1 x 4 rts -> one 2-bank psum; hi-half
                    relu+bias on ACT -> r_hi; out_cb(ps, r_hi) pools."""
                    ps = psA.tile([128, 2, 512], F32, tag="psA", name="psA")
                    for bi in range(2):
                        ovl = overlaps(conv, b0 + bi)
                        for i, (s, dlt) in enumerate(ovl):
                            nc.tensor.matmul(
                                ps[:, bi, :],
                                lhsT=Tv[d2i[conv][dlt]][:], rhs=srcs(s),
                                start=(i == 0), stop=(i == len(ovl) - 1))
                    r_hi = cnnp.tile([64, 2, 512], BF16, tag="r_hi", name="r_hi")
                    nc.scalar.activation(r_hi[:], ps[64:128], AF.Relu,
                                         bias=bias[64:128, :])
                    out_cb(ps, r_hi)

                # conv0 -> pooled0 [64, 8, 4, 128]
                pooled0 = cnnp.tile([64, NB0, 4, 128], BF16, tag="pooled0",
                                    name="pooled0")
                for b0_ in range(0, NB0, 2):
                    def p0(ps, r_hi, b0_=b0_):
                        nc.vector.scalar_tensor_tensor(
                            pooled0[:, b0_:b0_ + 2, :, :],
                            in0=ps[0:64].rearrange("p a (j r) -> p a j r", j=4),
                            scalar=b0e[0:64, :],
                            in1=r_hi[:].rearrange("p a (j r) -> p a j r", j=4),
                            op0=OP.add, op1=OP.max)
                    conv_unit(0, T0v, lambda s: xt4[:, s, :, :], b0e, b0_, p0)

                # conv1 -> pooled1 [64, 16, 4, 128]
                pooled1 = cnnp.tile([64, NB1, 4, 128], BF16, tag="pooled1",
                                    name="pooled1")
                for b0_ in range(0, NB1, 2):
                    def p1(ps, r_hi, b0_=b0_):
                        nc.vector.scalar_tensor_tensor(
                            pooled1[:, b0_:b0_ + 2, :, :],
                            in0=ps[0:64].rearrange("p a (j r) -> p a j r", j=4),
                            scalar=b1e[0:64, :],
                            in1=r_hi[:].rearrange("p a (j r) -> p a j r", j=4),
                            op0=OP.add, op1=OP.max)
                    conv_unit(1, T1v, lambda s: pooled0[:, s, :, :], b1e, b0_, p1)

                # conv2 -> act3 [128, 16, 4, 128]; parity -> partition half
                act3 = cnnp.tile([128, 16, 4, 128], BF16, tag="act3", name="act3")
                for b0_ in range(0, NB2, 2):
                    def p2(ps, r_hi, b0_=b0_):
                        ch = b0_ // 2
                        nc.vector.scalar_tensor_tensor(
                            act3[0:64, ch, :, :],
                            in0=ps[0:64, 0].rearrange("p (j r) -> p j r", j=4),
                            scalar=b2e[0:64, :],
                            in1=r_hi[:, 0].rearrange("p (j r) -> p j r", j=4),
                            op0=OP.add, op1=OP.max)
                        nc.vector.scalar_tensor_tensor(
                            act3[64:128, ch, :, :],
                            in0=ps[0:64, 1].rearrange("p (j r) -> p j r", j=4),
                            scalar=b2e[0:64, :],
                            in1=r_hi[:, 1].rearrange("p (j r) -> p j r", j=4),
                            op0=OP.add, op1=OP.max)
                    conv_unit(2, T2v, lambda s: pooled1[:, s, :, :], b2e, b0_, p2)

                # embed + bias + relu + pe -> xn_rm / xn_fm per rt
                for j, rt in enumerate(rts):
                    pse = psB.tile([128, 512], F32, tag="psB", name="psB")
                    for c in range(16):
                        nc.tensor.matmul(pse[:, 0:D], lhsT=act3[:, c, j, :],
                                         rhs=We[c][:],
                                         start=(c == 0), stop=False)
                    nc.tensor.matmul(pse[:, 0:D], lhsT=ones_bf[:, 0:128],
                                     rhs=eb_row[:], start=False, stop=True)
                    nc.vector.scalar_tensor_tensor(
                        xn_rm[:, rt, :], in0=pse[:, 0:D], scalar=0.0,
                        in1=pe_rm[:], op0=OP.max, op1=OP.add)
                    psx = psC.tile([128, 3, 128], BF16, tag="psC", name="psC2")
                    for c in range(3):
                        nc.tensor.transpose(psx[:, c, :],
                                            xn_rm[:, rt, c * 128:(c + 1) * 128],
                                            idn_b[:])
                    if rt % 2:
                        nc.vector.tensor_copy(
                            xn_fm[:, :, rt * 128:(rt + 1) * 128], psx[:])
                    else:
                        nc.scalar.copy(xn_fm[:, :, rt * 128:(rt + 1) * 128],
                                       psx[:])

        # transformer-only state (own pool: allocated after CNN pools
        # release so it reuses their SBUF space)
        tstate = ctx.enter_context(tc.tile_pool(name="tstate", bufs=1))
        o_fm = tstate.tile([128, 3, R], BF16, tag="o_fm", name="o_fm")
        h1 = tstate.tile([128, 12, R], BF16, tag="h1", name="h1")
        qc_t = tstate.tile([128, 4, R], BF16, tag="qc_t", name="qc_t")
        kc_t = tstate.tile([128, 4, R], BF16, tag="kc_t", name="kc_t")
        qo_t = tstate.tile([64, 4, R], BF16, tag="qo_t", name="qo_t")
        ko_t = tstate.tile([64, 4, R], BF16, tag="ko_t", name="ko_t")

        # ------------------------------------------------------- transformer
        work = ctx.enter_context(tc.tile_pool(name="work", bufs=3))
        lnw = ctx.enter_context(tc.tile_pool(name="lnw", bufs=2))

        def ln_half(rts, px_of):
            """Half-batch layernorm: for rts, px_of(rt) emits matmuls into a
            fresh psB and returns it (pre-LN sum incl. residual+bias).
            Writes xn_rm / xn_fm."""
            x1s = {}
            st = lnw.tile([128, 8, 2], F32, tag="st", name="st")  # s1, s2
            for j, rt in enumerate(rts):
                px = px_of(rt)
                x1 = lnw.tile([128, D], BF16, tag=f"x1_{j}", name=f"x1_{j}")
                # copy + running sum  (rotate ACT / Pool)
                if j % 2 == 0:
                    nc.scalar.activation(x1[:], px[:, 0:D], AF.Identity,
                                         accum_out=st[:, j, 0:1])
                else:
                    nc.vector.tensor_scalar(x1[:], px[:, 0:D], 0.0, 0.0, OP.add,
                                            OP.add, accum_out=st[:, j, 0:1])
                xsq = work.tile([128, D], BF16, tag="xsq", name="xsq")
                nc.vector.scalar_tensor_tensor(
                    xsq[:], in0=x1[:], scalar=0.0, in1=x1[:],
                    op0=OP.add, op1=OP.mult, accum_out=st[:, j, 1:2])
                x1s[rt] = x1
            # mean = s1/D; var = s2/D - mean^2 ; rstd = 1/sqrt(var+eps)
            nb = len(rts)
            mean = lnw.tile([128, 8], F32, tag="mean", name="mean")
            var = lnw.tile([128, 8], F32, tag="var", name="var")
            nc.vector.tensor_scalar(mean[:, 0:nb], st[:, 0:nb, 0], 1.0 / D, None,
                                    OP.mult)
            msq = work.tile([128, 8], F32, tag="msq", name="msq")
            nc.vector.tensor_tensor(msq[:, 0:nb], mean[:, 0:nb], mean[:, 0:nb],
                                    OP.mult)
            nc.vector.scalar_tensor_tensor(
                var[:, 0:nb], in0=st[:, 0:nb, 1], scalar=1.0 / D, in1=msq[:, 0:nb],
                op0=OP.mult, op1=OP.subtract)
            sd = work.tile([128, 8], F32, tag="sd", name="sd")
            nc.scalar.activation(sd[:, 0:nb], var[:, 0:nb], AF.Sqrt, bias=epsc[:])
            rstd = lnw.tile([128, 8], F32, tag="rstd", name="rstd")
            nc.vector.reciprocal(rstd[:, 0:nb], sd[:, 0:nb])
            mrs = lnw.tile([128, 8], F32, tag="mrs", name="mrs")
            nc.vector.scalar_tensor_tensor(
                mrs[:, 0:nb], in0=mean[:, 0:nb], scalar=-1.0, in1=rstd[:, 0:nb],
                op0=OP.mult, op1=OP.mult)
            for j, rt in enumerate(rts):
                nc.vector.tensor_scalar(
                    xn_rm[:, rt, :], x1s[rt][:], rstd[:, j:j + 1],
                    mrs[:, j:j + 1], OP.mult, OP.add)
                pst = psC.tile([128, 3, 128], BF16, tag="psC", name="psC")
                for c in range(3):
                    nc.tensor.transpose(pst[:, c, :],
                                        xn_rm[:, rt, c * 128:(c + 1) * 128],
                                        idn_b[:])
                if j % 2 == 0:
                    nc.vector.tensor_copy(
                        xn_fm[:, :, rt * 128:(rt + 1) * 128], pst[:])
                else:
                    nc.scalar.copy(xn_fm[:, :, rt * 128:(rt + 1) * 128], pst[:])

        for lyr in range(n_layers):
            wl, bl, rl, dga, dgf = cur
            if lyr + 1 < n_layers:
                cur = load_layer(lyr + 1)

            def wv_(c):   # [128, 384] views into the blob
                return wl[:, c * CSEC + 1024:c * CSEC + 1408]
            def wo_(c):
                return wl[:, c * CSEC + 1408:c * CSEC + 1792]
            def w1_(c):
                return wl[:, c * CSEC + 1792:c * CSEC + 3328]
            def w2_(dc):
                return wl[:, 3 * CSEC + dc * 384:3 * CSEC + (dc + 1) * 384]

            if phase < 2:
                continue
            # batched Q/K into head-padded feature-major tiles (4 out-chunks)
            for pi, (dstc, woff, bcol) in enumerate(
                    ((qc_t, 0, 0), (kc_t, 512, 4))):
                for oc in range(4):
                    pq = psA.tile([128, 2, 512], F32, tag="psA", name="psA")
                    for hf in range(2):
                        for c in range(3):
                            nc.tensor.matmul(
                                pq[:, hf, :],
                                lhsT=wl[:, c * CSEC + woff + oc * 128:
                                        c * CSEC + woff + (oc + 1) * 128],
                                rhs=xn_fm[:, c, hf * 512:(hf + 1) * 512],
                                start=(c == 0), stop=(c == 2))
                    eng = (nc.scalar, nc.vector)[(pi * 4 + oc) % 2]
                    if eng is nc.scalar:
                        nc.scalar.activation(dstc[:, oc, :],
                                             pq[:].rearrange("p a b -> p (a b)"),
                                             AF.Identity,
                                             bias=bl[:, bcol + oc:bcol + oc + 1])
                    else:
                        eng.tensor_scalar(dstc[:, oc, :],
                                          pq[:].rearrange("p a b -> p (a b)"),
                                          bl[:, bcol + oc:bcol + oc + 1], None,
                                          OP.add)
            nc.sync.dma_start(qo_t[:], qc_t[64:128, :, :])
            nc.sync.dma_start(ko_t[:], kc_t[64:128, :, :])

            # per-sample attention
            for n in range(RPC):
                cs = slice(n * 128, (n + 1) * 128)
                if phase < 3:
                    break
                # V (+ bias fold) -> v_ext with ones column per head
                pv = psB.tile([128, 512], F32, tag="psB", name="psB")
                for c in range(3):
                    nc.tensor.matmul(pv[:, 0:D], lhsT=xn_fm[:, c, cs], rhs=wv_(c),
                                     start=(c == 0), stop=False)
                nc.tensor.matmul(pv[:, 0:D], lhsT=ones_bf[:, 0:128],
                                 rhs=rl[:, 0:D], start=False, stop=True)
                v_ext = work.tile([128, H, E + 1], BF16, tag="v_ext", name="v_ext")
                nc.vector.memset(v_ext[:, :, E:E + 1], 1.0)
                nc.vector.tensor_copy(
                    v_ext[:, :, 0:E],
                    pv[:, 0:D].rearrange("p (h e) -> p h e", h=H))
                if phase < 4:
                    continue

                # scores S^T = K^T Q per head (all operands base partition 0)
                pss = psA.tile([128, 2, 512], F32, tag="psA", name="psA")
                for h in range(H):
                    kk = kc_t if h % 2 == 0 else ko_t
                    qq = qc_t if h % 2 == 0 else qo_t
                    nc.tensor.matmul(
                        pss[:, h // 4, (h % 4) * 128:(h % 4 + 1) * 128],
                        lhsT=kk[0:E, h // 2, cs],
                        rhs=qq[0:E, h // 2, cs],
                        start=True, stop=True)
                es16 = work.tile([128, H, 128], BF16, tag="es16", name="es16")
                if split_exp:
                    for hf in range(2):
                        nc.scalar.activation(
                            es16[:, hf * 4:(hf + 1) * 4, :],
                            pss[:, hf].rearrange("p (b r) -> p b r", b=4),
                            AF.Exp, scale=TEMP)
                else:
                    nc.scalar.activation(
                        es16[:], pss[:].rearrange("p a (b r) -> p (a b) r", b=4),
                        AF.Exp, scale=TEMP)
                if phase < 5:
                    continue

                # A·[V|1] -> per-head 49-col groups: o unnormalized + row sums
                pso = psB.tile([128, 512], F32, tag="psB", name="psO")
                for h in range(H):
                    nc.tensor.matmul(pso[:, h * 49:(h + 1) * 49],
                                     lhsT=es16[:, h, :], rhs=v_ext[:, h, :],
                                     start=True, stop=True)
                pso_v = pso[:, 0:392].rearrange("p (h e) -> p h e", h=H)
                rr = work.tile([128, H], F32, tag="rr", name="rr")
                nc.vector.reciprocal(rr[:], pso_v[:, :, E])
                o_rm = work.tile([128, D], BF16, tag="o_rm", name="o_rm")
                rrb = AP(rr.tensor, rr.offset, [list(rr.ap[0]), [1, H], [0, E]])
                nc.vector.tensor_tensor(o_rm[:].rearrange("p (h e) -> p h e", h=H),
                                        pso_v[:, :, 0:E], rrb, OP.mult)
                ps = psC.tile([128, 3, 128], BF16, tag="psC", name="psC")
                for c in range(3):
                    nc.tensor.transpose(ps[:, c, :], o_rm[:, c * 128:(c + 1) * 128],
                                        idn_b[:])
                if n % 2:
                    nc.vector.tensor_copy(o_fm[:, :, cs], ps[:])
                else:
                    nc.scalar.copy(o_fm[:, :, cs], ps[:])

            if phase < 6:
                continue

            # LN1: px = o@Wo + bo' + resid(xn)
            def attn_px(rt):
                cs = slice(rt * 128, (rt + 1) * 128)
                px = psB.tile([128, 512], F32, tag="psB", name="psB")
                for c in range(3):
                    nc.tensor.matmul(px[:, 0:D], lhsT=o_fm[:, c, cs], rhs=wo_(c),
                                     start=(c == 0), stop=False)
                nc.tensor.matmul(px[:, 0:D], lhsT=ones_bf[:, 0:128],
                                 rhs=rl[:, D:2 * D], start=False, stop=False)
                if dga is None:
                    nc.tensor.matmul(px[:, 0:D], lhsT=idn_b[:], rhs=xn_rm[:, rt, :],
                                     start=False, stop=True)
                else:
                    dv = dga[:].rearrange("p (c d) -> p c d", c=3)
                    for c in range(3):
                        nc.tensor.matmul(px[:, 0:D], lhsT=xn_fm[:, c, cs],
                                         rhs=dv[:, c], start=False, stop=(c == 2))
                return px

            ln_half(range(0, 4), attn_px)
            ln_half(range(4, 8), attn_px)

            if phase < 7:
                continue
            # FFN: h1 = relu(xn @ W1' + b1')
            for dc in range(12):
                ph = psA.tile([128, 2, 512], F32, tag="psA", name="psA")
                for hf in range(2):
                    for c in range(3):
                        nc.tensor.matmul(
                            ph[:, hf, :],
                            lhsT=w1_(c)[:, dc * 128:(dc + 1) * 128],
                            rhs=xn_fm[:, c, hf * 512:(hf + 1) * 512],
                            start=(c == 0), stop=(c == 2))
                eng = (nc.scalar, nc.vector)[dc % 2]
                if eng is nc.scalar:
                    nc.scalar.activation(h1[:, dc, :],
                                         ph[:].rearrange("p a b -> p (a b)"),
                                         AF.Relu, bias=bl[:, 8 + dc:9 + dc])
                else:
                    eng.tensor_scalar(h1[:, dc, :],
                                      ph[:].rearrange("p a b -> p (a b)"),
                                      bl[:, 8 + dc:9 + dc], 0.0, OP.add, OP.max)
            if phase < 8:
                continue

            # LN2: px = h1@W2 + b2' + resid(xn)
            def ffn_px(rt):
                cs = slice(rt * 128, (rt + 1) * 128)
                px = psB.tile([128, 512], F32, tag="psB", name="psB")
                for dc in range(12):
                    nc.tensor.matmul(px[:, 0:D], lhsT=h1[:, dc, cs], rhs=w2_(dc),
                                     start=(dc == 0), stop=False)
                nc.tensor.matmul(px[:, 0:D], lhsT=ones_bf[:, 0:128],
                                 rhs=rl[:, 2 * D:3 * D], start=False, stop=False)
                if dgf is None:
                    nc.tensor.matmul(px[:, 0:D], lhsT=idn_b[:], rhs=xn_rm[:, rt, :],
                                     start=False, stop=True)
                else:
                    dv = dgf[:].rearrange("p (c d) -> p c d", c=3)
                    for c in range(3):
                        nc.tensor.matmul(px[:, 0:D], lhsT=xn_fm[:, c, cs],
                                         rhs=dv[:, c], start=False, stop=(c == 2))
                return px

            ln_half(range(0, 4), ffn_px)
            ln_half(range(4, 8), ffn_px)

        # ------------------------------------------------------- head
        outsb = state.tile([1, RPC], F32, tag="outsb", name="outsb")
        for n in range(RPC):
            pm = psC.tile([128, 3], F32, tag="psC", name="psCh")
            for c in range(3):
                nc.tensor.matmul(pm[:, c:c + 1],
                                 lhsT=xn_rm[:, n, c * 128:(c + 1) * 128],
                                 rhs=onesL[:], start=True, stop=True)
            tm = work.tile([128, 3], F32, tag="tm", name="tm")
            nc.scalar.copy(tm[:], pm[:])
            pc = psC.tile([1, 8], F32, tag="psC", name="psCh2")
            for c in range(3):
                nc.tensor.matmul(pc[:, 0:1], lhsT=tm[:, c:c + 1],
                                 rhs=clsw[:, c:c + 1],
                                 start=(c == 0), stop=(c == 2))
            nc.scalar.activation(outsb[:, n:n + 1], pc[:, 0:1], AF.Identity,
                                 bias=clsb[:])
        nc.sync.dma_start(y_d[:].rearrange("a b -> b a"), outsb[:])

    if do_compile:
        nc.compile()
    return nc


_PROG = {}


def _get_prog(gp_ident=None, n_layers=NL, phase=99):
    key = (tuple(gp_ident) if gp_ident else None, n_layers, phase)
    if key not in _PROG:
        _PROG[key] = build_program(gp_ident, n_layers=n_layers, phase=phase)
    return _PROG[key]


def _in_maps(inputs):
    shared = host_prep(inputs)
    gp_ident = shared.pop("gp_identity")
    x = np.asarray(inputs["x"], np.float32)  # (64, 128, 256)
    in_maps = []
    for c in range(NCORES):
        m = dict(shared)
        # [128 tokens, rt, W] per core
        xc = x[c * RPC:(c + 1) * RPC]              # (8, 128, 256)
        m["xc"] = np.ascontiguousarray(xc.transpose(1, 0, 2))
        in_maps.append(m)
    return in_maps, gp_ident


def kernel(**inputs):
    in_maps, gp_ident = _in_maps(inputs)
    nc = _get_prog(gp_ident)
    res = run_bass_kernel_spmd(nc, in_maps, core_ids=list(range(NCORES)))
    out = np.concatenate([res.results[c]["yc"] for c in range(NCORES)], axis=0)
    return out.astype(np.float32)


def timed_run(inputs, iters=32):
    """Estimate per-execution HW time by chaining NEFF executions.

    No NTFF hook is available through this axon tunnel, so true HW exec
    time can't be read from a profile.  Instead we chain k executions
    (each iteration's outputs feed the next call's operands, forcing
    device-side serialization while dispatch pipelines) and report the
    marginal wall time per added execution:  (t_k - t_1) / (k - 1).
    This subtracts the fixed per-dispatch tunnel overhead (~80 ms) that
    would otherwise swamp the measurement.  Returns ns.
    """
    import time
    import jax
    from jax.experimental.shard_map import shard_map
    from jax.sharding import Mesh, NamedSharding, PartitionSpec
    from concourse import bass2jax, mybir as mb

    in_maps, gp_ident = _in_maps(inputs)
    nc = _get_prog(gp_ident)
    bass2jax.install_neuronx_cc_hook()
    partition_name = nc.partition_id_tensor.name if nc.partition_id_tensor else None
    in_names, out_names, out_avals, zero_outs = [], [], [], []
    for alloc in nc.m.functions[0].allocations:
        if not isinstance(alloc, mb.MemoryLocationSet):
            continue
        name = alloc.memorylocations[0].name
        if alloc.kind == "ExternalInput":
            if name != partition_name:
                in_names.append(name)
        elif alloc.kind == "ExternalOutput":
            shape = tuple(alloc.tensor_shape)
            dtype = mb.dt.np(alloc.dtype)
            out_avals.append(jax.core.ShapedArray(shape, dtype))
            out_names.append(name)
            zero_outs.append(np.zeros(shape, dtype))
    n_params, n_outs = len(in_names), len(out_avals)
    all_in = list(in_names) + list(out_names)
    if partition_name is not None:
        all_in.append(partition_name)

    def _body(*args):
        ins = list(args[:n_params])
        outs = list(args[n_params:])
        operands = ins + outs
        if partition_name is not None:
            operands = operands + [bass2jax.partition_id_tensor()]
        outs = list(bass2jax._bass_exec_p.bind(
            *operands, out_avals=tuple(out_avals), in_names=tuple(all_in),
            out_names=tuple(out_names), lowering_input_output_aliases=(),
            sim_require_finite=True, sim_require_nnan=True, nc=nc))
        return tuple(outs)

    devices = jax.devices()[:NCORES]
    mesh = Mesh(np.asarray(devices), ("core",))
    shard = NamedSharding(mesh, PartitionSpec("core"))
    dev_in = [jax.device_put(
        np.concatenate([np.asarray(in_maps[c][nm]) for c in range(NCORES)], axis=0),
        shard) for nm in in_names]
    zsh = [np.zeros((NCORES * z.shape[0], *z.shape[1:]), z.dtype) for z in zero_outs]

    f = jax.jit(
        shard_map(_body, mesh=mesh,
                  in_specs=(PartitionSpec("core"),) * (n_params + n_outs),
                  out_specs=(PartitionSpec("core"),) * n_outs, check_rep=False),
        keep_unused=True)

    def run_chain(k):
        outs = [jax.device_put(z, shard) for z in zsh]
        jax.block_until_ready(outs)
        jax.block_until_ready(dev_in)
        t0 = time.perf_counter()
        for _ in range(k):
            outs = list(f(*dev_in, *outs))
        jax.block_until_ready(outs)
        return time.perf_counter() - t0

    run_chain(1)  # warm compile
    t1 = min(run_chain(1) for _ in range(6))
    tk = min(run_chain(iters) for _ in range(6))
    return int((tk - t1) / (iters - 1) * 1e9)




# revision 5
# speedup vs baseline: 1.8781x; 1.6107x over previous
"""Trainium2 Bass kernel v3 for nn_ClassificationModel.

Data parallel across 8 NeuronCores: batch N=64 -> 8 samples/core.

Differences vs v2 baseline:
  - All per-layer transformer weights packed into ONE [128, 13824] bf16 DMA
    (one HWDGE acquire per layer instead of ~40), double-buffered prefetch.
  - Scores read compact feature-major Q/K via partition-offset lhsT slices
    (heads 2/5 split across chunk boundaries -> 2 accumulating matmuls);
    the per-head 64-row spread DMAs are gone entirely.
  - LayerNorm affine (g, be) folded into adjacent weights host-side; device
    state is the *normalized* activation xn in bf16 (row- and feature-major).
    Residual enters the pre-LN sum as a single identity matmul.  LN stats
    come from accum_out sums (copy + square), apply is one tensor_scalar.
  - 2-PSUM-bank (free=1024) elementwise units for QK bias, scores exp and
    FFN relu; PSUM-consuming ops rotated across ACT/DVE/Pool for balance.
"""

import math
import sys

sys.path.insert(0, "/opt/trn_rl_repo")

import numpy as np
import ml_dtypes

import concourse.bass as bass
import concourse.mybir as mybir
import concourse.tile as tile
from concourse import bacc
from concourse.bass import AP
from concourse.bass_utils import run_bass_kernel_spmd

BF = ml_dtypes.bfloat16
F32 = mybir.dt.float32
BF16 = mybir.dt.bfloat16
AX = mybir.AxisListType
OP = mybir.AluOpType
AF = mybir.ActivationFunctionType

# model dims
N, L, W = 64, 128, 256
D, H, NL, DFF = 384, 8, 4, 1536
E = D // H  # 48
CH = [1, 4, 16, 64]
K = 7
NCORES = 8
RPC = N // NCORES          # samples per core = 8
R = RPC * L                # rows per core = 1024
TEMP = 1.0 / math.sqrt(E)
EPS = 1e-5

# conv block sizes (output positions per Toeplitz block)
B0, B1, B2 = 32, 8, 2
NB0, NB1, NB2 = 256 // B0, 128 // B1, 64 // B2  # 8, 16, 32

# packed per-layer weight blob column offsets (bf16, [128, WCOLS])
# chunk c in 0..2: Wq(512, head-padded) Wk(512) Wv(384) Wo(384) W1(1536)
# at c*3328; then W2: 12 chunks x 384 at 9984.
# Q/K output features are head-padded: head h -> rows 64h..64h+47 of 512,
# so per-head score matmuls read base partitions 0/64 (hw constraint).
CSEC = 512 + 512 + 384 + 384 + 1536  # 3328
PSB_BUFS, PSC_BUFS = 2, 2
LN_HB = 4
WCOLS = 3 * CSEC + 12 * 384          # 14592


# ---------------------------------------------------------------------------
# host-side weight preparation
# ---------------------------------------------------------------------------

def _pe_np(l, d):
    pos = np.arange(l)[:, None].astype(np.float32)
    i = np.arange(d // 2)[None, :].astype(np.float32)
    ang = pos / np.power(10000.0, 2.0 * i / d)
    pe = np.zeros((l, d), np.float32)
    pe[:, 0::2] = np.sin(ang)
    pe[:, 1::2] = np.cos(ang)
    return pe


# conv source-block overlap enumeration (shared host/device) -----------------

CONV_GEOM = {
    0: (B0, 128, 2, 1),
    1: (B1, 16, NB0, 4),
    2: (B2, 8, 8, 16),
}


def overlaps(conv, b):
    Bout, src_size, nsrc, _ = CONV_GEOM[conv]
    w0, w1 = Bout * b - 3, Bout * b + Bout + 3
    res = []
    for s in range(nsrc):
        lo, hi = s * src_size, (s + 1) * src_size
        if max(w0, lo) < min(w1, hi):
            res.append((s, lo - Bout * b))
    return res


def conv_deltas(conv):
    nb = {0: NB0, 1: NB1, 2: NB2}[conv]
    return sorted({d for b in range(nb) for _, d in overlaps(conv, b)})


def _m_layout(conv, h, co):
    if conv == 0:
        return (h & 1) * 64 + (h >> 1) * 4 + co
    if conv == 1:
        return (h & 1) * 64 + (h >> 1) * 16 + co
    return h * 64 + co


def _toeplitz_variants(conv, w):
    Bout, src_size, _, nch = CONV_GEOM[conv]
    cout = w.shape[0]
    ds = conv_deltas(conv)
    T = np.zeros((len(ds), src_size * nch, 128), np.float32)
    for vi, delta in enumerate(ds):
        for hp in range(src_size):
            for h in range(Bout):
                k = delta + hp - h + 3
                if 0 <= k < K:
                    for co in range(cout):
                        for ci in range(nch):
                            T[vi, hp * nch + ci, _m_layout(conv, h, co)] = w[co, ci, k]
    return T


def host_prep(inp):
    d = {}
    f32 = np.float32
    d["T0"] = _toeplitz_variants(0, np.asarray(inp["conv_w0"], f32)).astype(BF)
    d["T1"] = _toeplitz_variants(1, np.asarray(inp["conv_w1"], f32)).astype(BF)
    d["T2"] = _toeplitz_variants(2, np.asarray(inp["conv_w2"], f32)).astype(BF)
    b0, b1c, b2c = (np.asarray(inp[f"conv_b{i}"], f32) for i in range(3))
    p = np.arange(128)
    d["b0e"] = b0[p % 4].reshape(128, 1)
    d["b1e"] = b1c[p % 16].reshape(128, 1)
    d["b2e"] = b2c[p % 64].reshape(128, 1)

    # embed: We_r[c, p, :] = embed_w[(p%64)*32 + 2c + p//64, :]
    ew = np.asarray(inp["embed_w"], f32)  # (2048, 384)
    We_r = np.zeros((16, 128, D), f32)
    for c in range(16):
        for pi in range(128):
            We_r[c, pi] = ew[(pi % 64) * 32 + 2 * c + pi // 64]
    d["We_r"] = We_r.astype(BF)
    d["eb_row"] = np.asarray(inp["embed_b"], f32).reshape(1, D).astype(BF)
    d["pe_rm"] = _pe_np(L, D)

    g1 = np.asarray(inp["g1"], f32)
    be1 = np.asarray(inp["be1"], f32)
    g2 = np.asarray(inp["g2"], f32)
    be2 = np.asarray(inp["be2"], f32)

    # pending affine entering each layer's attention block
    gp = np.stack([np.ones(D, f32) if l == 0 else g2[l - 1] for l in range(NL)])
    bp = np.stack([np.zeros(D, f32) if l == 0 else be2[l - 1] for l in range(NL)])
    d["gp_identity"] = [bool(np.all(gp[l] == 1.0)) for l in range(NL)] + \
                       [bool(np.all(g1[l] == 1.0)) for l in range(NL)]

    Wq = np.asarray(inp["Wq"], f32)
    Wk = np.asarray(inp["Wk"], f32)
    Wv = np.asarray(inp["Wv"], f32)
    Wo = np.asarray(inp["Wo"], f32)
    W1 = np.asarray(inp["W1"], f32)
    W2 = np.asarray(inp["W2"], f32)
    bq = np.asarray(inp["bq"], f32)
    bk = np.asarray(inp["bk"], f32)
    bv = np.asarray(inp["bv"], f32)
    bo = np.asarray(inp["bo"], f32)
    b1 = np.asarray(inp["b1"], f32)
    b2 = np.asarray(inp["b2"], f32)

    # fold pending affines into weights/biases
    WqF = gp[:, :, None] * Wq
    WkF = gp[:, :, None] * Wk
    WvF = gp[:, :, None] * Wv
    bqF = bq + np.einsum("ld,lde->le", bp, Wq)
    bkF = bk + np.einsum("ld,lde->le", bp, Wk)
    bvF = bv + np.einsum("ld,lde->le", bp, Wv)
    W1F = g1[:, :, None] * W1
    b1F = b1 + np.einsum("ld,lde->le", be1, W1)
    boF = bo + bp          # LN1 pre-sum bias includes pending be
    b2F = b2 + be1         # LN2 pre-sum bias includes LN1's be

    # diag blocks for non-identity pending g (3 chunks of [128, 384] each)
    def diag_chunks(g):
        out = np.zeros((128, 3 * D), f32)
        for c in range(3):
            for i in range(128):
                out[i, c * D + c * 128 + i] = g[c * 128 + i]
        return out
    d["dg_attn"] = np.stack([diag_chunks(gp[l]) for l in range(NL)]).astype(BF)
    d["dg_ffn"] = np.stack([diag_chunks(g1[l]) for l in range(NL)]).astype(BF)

    # head-pad Q/K output features: head h -> cols 64h..64h+47 of 512
    def head_pad_w(w):  # (NL, 384, 384) -> (NL, 384, 512)
        out = np.zeros((NL, D, 512), f32)
        for h in range(H):
            out[:, :, 64 * h:64 * h + E] = w[:, :, E * h:E * (h + 1)]
        return out

    def head_pad_b(b):  # (NL, 384) -> (NL, 512)
        out = np.zeros((NL, 512), f32)
        for h in range(H):
            out[:, 64 * h:64 * h + E] = b[:, E * h:E * (h + 1)]
        return out

    WqP, WkP = head_pad_w(WqF), head_pad_w(WkF)
    bqP, bkP = head_pad_b(bqF), head_pad_b(bkF)

    # mega weight blob per layer
    WL = np.zeros((NL, 128, WCOLS), f32)
    for l in range(NL):
        for c in range(3):
            r = slice(c * 128, (c + 1) * 128)
            base = c * CSEC
            WL[l, :, base + 0:base + 512] = WqP[l][r]
            WL[l, :, base + 512:base + 1024] = WkP[l][r]
            WL[l, :, base + 1024:base + 1408] = WvF[l][r]
            WL[l, :, base + 1408:base + 1792] = Wo[l][r]
            WL[l, :, base + 1792:base + 3328] = W1F[l][r]
        for dc in range(12):
            WL[l, :, 3 * CSEC + dc * 384:3 * CSEC + (dc + 1) * 384] = \
                W2[l][dc * 128:(dc + 1) * 128]
    d["WL"] = WL.astype(BF)

    # per-layer f32 bias blob [128, 20]: bq (4 padded chunks), bk, b1r (12)
    BL = np.zeros((NL, 128, 20), f32)
    for oc in range(4):
        BL[:, :, oc] = bqP[:, oc * 128:(oc + 1) * 128]
        BL[:, :, 4 + oc] = bkP[:, oc * 128:(oc + 1) * 128]
    for l in range(NL):
        BL[l, :, 8:20] = b1F[l].reshape(12, 128).T
    d["BL"] = BL

    # per-layer bf16 rows blob [1, 1152]: bv | bo' | b2'
    RL = np.zeros((NL, 1, 3 * D), f32)
    RL[:, 0, 0:D] = bvF
    RL[:, 0, D:2 * D] = boF
    RL[:, 0, 2 * D:3 * D] = b2F
    d["RL"] = RL.astype(BF)

    d["idn_f"] = np.eye(128, dtype=f32)
    d["idn_b"] = np.eye(128, dtype=f32).astype(BF)
    d["onesL"] = np.full((128, 1), 1.0 / L, f32).astype(BF)
    # head fold: mean(g2[3]*xn + be2[3]) @ cls_w + cls_b
    cw = np.asarray(inp["cls_w"], f32)          # (384, 1)
    cb = np.asarray(inp["cls_b"], f32)          # (1,)
    cwF = (g2[NL - 1][:, None] * cw)
    cbF = cb + be2[NL - 1] @ cw
    d["clsw_r"] = cwF.reshape(3, 128).T.copy()  # (128, 3)
    d["clsb"] = cbF.reshape(1, 1)
    d["epsc"] = np.full((128, 1), EPS, f32)
    return d


# ---------------------------------------------------------------------------
# device program
# ---------------------------------------------------------------------------

def build_program(gp_ident=None, do_compile=True, n_layers=NL, phase=99, split_exp=False, pad_scores=False, even_only=False):
    if gp_ident is None:
        gp_ident = [True] * (2 * NL)
    nc = bacc.Bacc("TRN2", target_bir_lowering=False, debug=False)

    def dram_in(name, shape, dt=BF16):
        return nc.dram_tensor(name, list(shape), dt, kind="ExternalInput")

    x_d = dram_in("xc", (128, RPC, W), F32)
    nv0, nv1, nv2 = len(conv_deltas(0)), len(conv_deltas(1)), len(conv_deltas(2))
    T0_d = dram_in("T0", (nv0, 128, 128))
    T1_d = dram_in("T1", (nv1, 64, 128))
    T2_d = dram_in("T2", (nv2, 128, 128))
    b0e_d = dram_in("b0e", (128, 1), F32)
    b1e_d = dram_in("b1e", (128, 1), F32)
    b2e_d = dram_in("b2e", (128, 1), F32)
    We_d = dram_in("We_r", (16, 128, D))
    ebr_d = dram_in("eb_row", (1, D))
    pe_d = dram_in("pe_rm", (128, D), F32)
    WL_d = dram_in("WL", (NL, 128, WCOLS))
    BL_d = dram_in("BL", (NL, 128, 20), F32)
    RL_d = dram_in("RL", (NL, 1, 3 * D))
    dga_d = dram_in("dg_attn", (NL, 128, 3 * D))
    dgf_d = dram_in("dg_ffn", (NL, 128, 3 * D))
    idnf_d = dram_in("idn_f", (128, 128), F32)
    idnb_d = dram_in("idn_b", (128, 128))
    onesL_d = dram_in("onesL", (128, 1))
    clsw_d = dram_in("clsw_r", (128, 3), F32)
    eps_d = dram_in("epsc", (128, 1), F32)
    clsb_d = dram_in("clsb", (1, 1), F32)

    y_d = nc.dram_tensor("yc", [RPC, 1], F32, kind="ExternalOutput")

    from contextlib import ExitStack
    with tile.TileContext(nc) as tc, ExitStack() as ctx:
        const = ctx.enter_context(tc.tile_pool(name="const", bufs=1))
        state = ctx.enter_context(tc.tile_pool(name="state", bufs=1))
        psA = ctx.enter_context(tc.tile_pool(name="psA", bufs=2, space="PSUM"))
        psB = ctx.enter_context(tc.tile_pool(name="psB", bufs=PSB_BUFS, space="PSUM"))
        psC = ctx.enter_context(tc.tile_pool(name="psC", bufs=PSC_BUFS, space="PSUM"))

        # full input in one DMA (CNN-scoped pool, released before transformer)
        def load_const_in(pool, dram, shape, dt):
            nm = dram.name + "_sb"
            t = pool.tile(list(shape), dt, tag=nm, name=nm)
            nc.sync.dma_start(t[:], dram[:])
            return t

        const = ctx.enter_context(tc.tile_pool(name="const", bufs=1))
        state = ctx.enter_context(tc.tile_pool(name="state", bufs=1))
        idn_f = load_const_in(const, idnf_d, (128, 128), F32)
        idn_b = load_const_in(const, idnb_d, (128, 128), BF16)
        onesL = load_const_in(const, onesL_d, (128, 1), BF16)
        clsw = load_const_in(const, clsw_d, (128, 3), F32)
        epsc = load_const_in(const, eps_d, (128, 1), F32)
        clsb = load_const_in(const, clsb_d, (1, 1), F32)
        ones_bf = const.tile([1, 512], BF16, tag="ones_bf", name="ones_bf")
        nc.vector.memset(ones_bf[:], 1.0)

        # persistent state written by CNN: normalized activations, bf16
        xn_rm = state.tile([128, RPC, D], BF16, tag="xn_rm", name="xn_rm")
        xn_fm = state.tile([128, 3, R], BF16, tag="xn_fm", name="xn_fm")

        # transformer weights pool must outlive the CNN block (prefetch L0)
        wpool = ctx.enter_context(tc.tile_pool(name="wpool", bufs=2))

        WLs, BLs, RLs, DGAs, DGFs = [], [], [], [], []
        def load_layer(l):
            wl = wpool.tile([128, WCOLS], BF16, tag="WL", name=f"WL{l}")
            nc.sync.dma_start(wl[:], WL_d[l])
            bl = wpool.tile([128, 20], F32, tag="BL", name=f"BL{l}")
            nc.sync.dma_start(bl[:], BL_d[l])
            rl = wpool.tile([1, 3 * D], BF16, tag="RL", name=f"RL{l}")
            nc.sync.dma_start(rl[:], RL_d[l])
            dga = dgf = None
            if not gp_ident[l]:
                dga = wpool.tile([128, 3 * D], BF16, tag="DGA", name=f"DGA{l}")
                nc.sync.dma_start(dga[:], dga_d[l])
            if not gp_ident[NL + l]:
                dgf = wpool.tile([128, 3 * D], BF16, tag="DGF", name=f"DGF{l}")
                nc.sync.dma_start(dgf[:], dgf_d[l])
            return (wl, bl, rl, dga, dgf)

        cur = load_layer(0)

        # ------------------------------------------------------- CNN + embed
        # 4 row-tiles per group: conv matmuls move 512 cols (4 rts) at once
        with tc.tile_pool(name="cnnc", bufs=1) as cnnc, \
                tc.tile_pool(name="cnn", bufs=2) as cnnp:
            x_all = cnnc.tile([128, RPC, W], F32, tag="x_all", name="x_all")
            nc.sync.dma_start(x_all[:], x_d[:])
            T0v, T1v, T2v = [], [], []
            for conv, (dst, dram, npart) in enumerate(
                    ((T0v, T0_d, 128), (T1v, T1_d, 64), (T2v, T2_d, 128))):
                for vi in range(len(conv_deltas(conv))):
                    t = cnnc.tile([npart, 128], BF16, tag=f"Tv{conv}_{vi}",
                                  name=f"Tv{conv}_{vi}")
                    nc.sync.dma_start(t[:], dram[vi])
                    dst.append(t)
            d2i = [{d: i for i, d in enumerate(conv_deltas(c))} for c in range(3)]
            b0e = load_const_in(cnnc, b0e_d, (128, 1), F32)
            b1e = load_const_in(cnnc, b1e_d, (128, 1), F32)
            b2e = load_const_in(cnnc, b2e_d, (128, 1), F32)
            eb_row = load_const_in(cnnc, ebr_d, (1, D), BF16)
            pe_rm = load_const_in(cnnc, pe_d, (128, D), F32)
            We = []
            for c in range(16):
                t = cnnc.tile([128, D], BF16, tag=f"We{c}", name=f"We{c}")
                nc.sync.dma_start(t[:], We_d[c])
                We.append(t)

            for g in range(2):
                rts = range(g * 4, (g + 1) * 4)
                # transpose x: per rt, both halves -> xt4 [128, half, rt, 128]
                xt4 = cnnp.tile([128, 2, 4, 128], BF16, tag="xt4", name="xt4")
                for j, rt in enumerate(rts):
                    psx = psC.tile([128, 3, 128], F32, tag="psC", name="psC")
                    for half in range(2):
                        nc.tensor.transpose(
                            psx[:, half, :],
                            x_all[:, rt, half * 128:(half + 1) * 128], idn_f[:])
                    if j % 2 == 0:
                        nc.scalar.copy(xt4[:, :, j, :], psx[:, 0:2, :])
                    else:
                        nc.vector.tensor_copy(xt4[:, :, j, :], psx[:, 0:2, :])

                def conv_unit(conv, Tv, srcs, bias, b0, out_cb):
                    """blocks b0, b0+1 x 4 rts -> one 2-bank psum; hi-half
                    relu+bias on ACT -> r_hi; out_cb(ps, r_hi) pools."""
                    ps = psA.tile([128, 2, 512], F32, tag="psA", name="psA")
                    for bi in range(2):
                        ovl = overlaps(conv, b0 + bi)
                        for i, (s, dlt) in enumerate(ovl):
                            nc.tensor.matmul(
                                ps[:, bi, :],
                                lhsT=Tv[d2i[conv][dlt]][:], rhs=srcs(s),
                                start=(i == 0), stop=(i == len(ovl) - 1))
                    r_hi = cnnp.tile([64, 2, 512], BF16, tag="r_hi", name="r_hi")
                    nc.scalar.activation(r_hi[:], ps[64:128], AF.Relu,
                                         bias=bias[64:128, :])
                    out_cb(ps, r_hi)

                # conv0 -> pooled0 [64, 8, 4, 128]
                pooled0 = cnnp.tile([64, NB0, 4, 128], BF16, tag="pooled0",
                                    name="pooled0")
                for b0_ in range(0, NB0, 2):
                    def p0(ps, r_hi, b0_=b0_):
                        nc.vector.scalar_tensor_tensor(
                            pooled0[:, b0_:b0_ + 2, :, :],
                            in0=ps[0:64].rearrange("p a (j r) -> p a j r", j=4),
                            scalar=b0e[0:64, :],
                            in1=r_hi[:].rearrange("p a (j r) -> p a j r", j=4),
                            op0=OP.add, op1=OP.max)
                    conv_unit(0, T0v, lambda s: xt4[:, s, :, :], b0e, b0_, p0)

                # conv1 -> pooled1 [128, 8, 4, 128]: conv1 block b's 4
                # pooled positions land at partitions 64*(b%2)+j*16+ci of
                # tile b//2 (8-pos/16-ch source tiles for conv2)
                pooled1 = cnnp.tile([128, NB1 // 2, 4, 128], BF16, tag="pooled1",
                                    name="pooled1")
                for b0_ in range(0, NB1, 2):
                    def p1(ps, r_hi, b0_=b0_):
                        for bi in range(2):
                            nc.vector.scalar_tensor_tensor(
                                pooled1[64 * bi:64 * bi + 64,
                                        b0_ // 2, :, :],
                                in0=ps[0:64, bi].rearrange(
                                    "p (j r) -> p j r", j=4),
                                scalar=b1e[0:64, :],
                                in1=r_hi[:, bi].rearrange(
                                    "p (j r) -> p j r", j=4),
                                op0=OP.add, op1=OP.max)
                    conv_unit(1, T1v, lambda s: pooled0[:, s, :, :], b1e, b0_, p1)

                # conv2 -> act3 [128, 16, 4, 128]; parity -> partition half
                act3 = cnnp.tile([128, 16, 4, 128], BF16, tag="act3", name="act3")
                for b0_ in range(0, NB2, 2):
                    def p2(ps, r_hi, b0_=b0_):
                        ch = b0_ // 2
                        nc.vector.scalar_tensor_tensor(
                            act3[0:64, ch, :, :],
                            in0=ps[0:64, 0].rearrange("p (j r) -> p j r", j=4),
                            scalar=b2e[0:64, :],
                            in1=r_hi[:, 0].rearrange("p (j r) -> p j r", j=4),
                            op0=OP.add, op1=OP.max)
                        nc.vector.scalar_tensor_tensor(
                            act3[64:128, ch, :, :],
                            in0=ps[0:64, 1].rearrange("p (j r) -> p j r", j=4),
                            scalar=b2e[0:64, :],
                            in1=r_hi[:, 1].rearrange("p (j r) -> p j r", j=4),
                            op0=OP.add, op1=OP.max)
                    conv_unit(2, T2v, lambda s: pooled1[:, s, :, :], b2e, b0_, p2)

                # embed + bias + relu + pe -> xn_rm / xn_fm per rt
                for j, rt in enumerate(rts):
                    pse = psB.tile([128, 512], F32, tag="psB", name="psB")
                    for c in range(16):
                        nc.tensor.matmul(pse[:, 0:D], lhsT=act3[:, c, j, :],
                                         rhs=We[c][:],
                                         start=(c == 0), stop=False)
                    nc.tensor.matmul(pse[:, 0:D], lhsT=ones_bf[:, 0:128],
                                     rhs=eb_row[:], start=False, stop=True)
                    nc.vector.scalar_tensor_tensor(
                        xn_rm[:, rt, :], in0=pse[:, 0:D], scalar=0.0,
                        in1=pe_rm[:], op0=OP.max, op1=OP.add)
                    psx = psC.tile([128, 3, 128], BF16, tag="psC", name="psC2")
                    for c in range(3):
                        nc.tensor.transpose(psx[:, c, :],
                                            xn_rm[:, rt, c * 128:(c + 1) * 128],
                                            idn_b[:])
                    if rt % 2:
                        nc.vector.tensor_copy(
                            xn_fm[:, :, rt * 128:(rt + 1) * 128], psx[:])
                    else:
                        nc.scalar.copy(xn_fm[:, :, rt * 128:(rt + 1) * 128],
                                       psx[:])

        # transformer-only state (own pool: allocated after CNN pools
        # release so it reuses their SBUF space)
        tstate = ctx.enter_context(tc.tile_pool(name="tstate", bufs=1))
        o_fm = tstate.tile([128, 3, R], BF16, tag="o_fm", name="o_fm")
        h1 = tstate.tile([128, 12, R], BF16, tag="h1", name="h1")
        qc_t = tstate.tile([128, 4, R], BF16, tag="qc_t", name="qc_t")
        kc_t = tstate.tile([128, 4, R], BF16, tag="kc_t", name="kc_t")
        qo_t = tstate.tile([64, 4, R], BF16, tag="qo_t", name="qo_t")
        ko_t = tstate.tile([64, 4, R], BF16, tag="ko_t", name="ko_t")

        # ------------------------------------------------------- transformer
        work = ctx.enter_context(tc.tile_pool(name="work", bufs=3))
        lnw = ctx.enter_context(tc.tile_pool(name="lnw", bufs=2))

        def ln_half(rts, px_of):
            """Half-batch layernorm: for rts, px_of(rt) emits matmuls into a
            fresh psB and returns it (pre-LN sum incl. residual+bias).
            Writes xn_rm / xn_fm."""
            x1s = {}
            st = lnw.tile([128, 8, 2], F32, tag="st", name="st")  # s1, s2
            for j, rt in enumerate(rts):
                px = px_of(rt)
                x1 = lnw.tile([128, D], BF16, tag=f"x1_{j}", name=f"x1_{j}")
                # copy + running sum  (rotate ACT / Pool)
                if j % 2 == 0:
                    nc.scalar.activation(x1[:], px[:, 0:D], AF.Identity,
                                         accum_out=st[:, j, 0:1])
                else:
                    nc.vector.tensor_scalar(x1[:], px[:, 0:D], 0.0, 0.0, OP.add,
                                            OP.add, accum_out=st[:, j, 0:1])
                xsq = work.tile([128, D], BF16, tag="xsq", name="xsq")
                nc.vector.scalar_tensor_tensor(
                    xsq[:], in0=x1[:], scalar=0.0, in1=x1[:],
                    op0=OP.add, op1=OP.mult, accum_out=st[:, j, 1:2])
                x1s[rt] = x1
            # mean = s1/D; var = s2/D - mean^2 ; rstd = 1/sqrt(var+eps)
            nb = len(rts)
            mean = lnw.tile([128, 8], F32, tag="mean", name="mean")
            var = lnw.tile([128, 8], F32, tag="var", name="var")
            nc.vector.tensor_scalar(mean[:, 0:nb], st[:, 0:nb, 0], 1.0 / D, None,
                                    OP.mult)
            msq = work.tile([128, 8], F32, tag="msq", name="msq")
            nc.vector.tensor_tensor(msq[:, 0:nb], mean[:, 0:nb], mean[:, 0:nb],
                                    OP.mult)
            nc.vector.scalar_tensor_tensor(
                var[:, 0:nb], in0=st[:, 0:nb, 1], scalar=1.0 / D, in1=msq[:, 0:nb],
                op0=OP.mult, op1=OP.subtract)
            sd = work.tile([128, 8], F32, tag="sd", name="sd")
            nc.scalar.activation(sd[:, 0:nb], var[:, 0:nb], AF.Sqrt, bias=epsc[:])
            rstd = lnw.tile([128, 8], F32, tag="rstd", name="rstd")
            nc.vector.reciprocal(rstd[:, 0:nb], sd[:, 0:nb])
            mrs = lnw.tile([128, 8], F32, tag="mrs", name="mrs")
            nc.vector.scalar_tensor_tensor(
                mrs[:, 0:nb], in0=mean[:, 0:nb], scalar=-1.0, in1=rstd[:, 0:nb],
                op0=OP.mult, op1=OP.mult)
            for j, rt in enumerate(rts):
                nc.vector.tensor_scalar(
                    xn_rm[:, rt, :], x1s[rt][:], rstd[:, j:j + 1],
                    mrs[:, j:j + 1], OP.mult, OP.add)
                pst = psC.tile([128, 3, 128], BF16, tag="psC", name="psC")
                for c in range(3):
                    nc.tensor.transpose(pst[:, c, :],
                                        xn_rm[:, rt, c * 128:(c + 1) * 128],
                                        idn_b[:])
                if j % 2 == 0:
                    nc.vector.tensor_copy(
                        xn_fm[:, :, rt * 128:(rt + 1) * 128], pst[:])
                else:
                    nc.scalar.copy(xn_fm[:, :, rt * 128:(rt + 1) * 128], pst[:])

        for lyr in range(n_layers):
            wl, bl, rl, dga, dgf = cur
            if lyr + 1 < n_layers:
                cur = load_layer(lyr + 1)

            def wv_(c):   # [128, 384] views into the blob
                return wl[:, c * CSEC + 1024:c * CSEC + 1408]
            def wo_(c):
                return wl[:, c * CSEC + 1408:c * CSEC + 1792]
            def w1_(c):
                return wl[:, c * CSEC + 1792:c * CSEC + 3328]
            def w2_(dc):
                return wl[:, 3 * CSEC + dc * 384:3 * CSEC + (dc + 1) * 384]

            if phase < 2:
                continue
            # batched Q/K into head-padded feature-major tiles (4 out-chunks)
            for pi, (dstc, woff, bcol) in enumerate(
                    ((qc_t, 0, 0), (kc_t, 512, 4))):
                for oc in range(4):
                    pq = psA.tile([128, 2, 512], F32, tag="psA", name="psA")
                    for hf in range(2):
                        for c in range(3):
                            nc.tensor.matmul(
                                pq[:, hf, :],
                                lhsT=wl[:, c * CSEC + woff + oc * 128:
                                        c * CSEC + woff + (oc + 1) * 128],
                                rhs=xn_fm[:, c, hf * 512:(hf + 1) * 512],
                                start=(c == 0), stop=(c == 2))
                    eng = (nc.scalar, nc.vector)[(pi * 4 + oc) % 2]
                    if eng is nc.scalar:
                        nc.scalar.activation(dstc[:, oc, :],
                                             pq[:].rearrange("p a b -> p (a b)"),
                                             AF.Identity,
                                             bias=bl[:, bcol + oc:bcol + oc + 1])
                    else:
                        eng.tensor_scalar(dstc[:, oc, :],
                                          pq[:].rearrange("p a b -> p (a b)"),
                                          bl[:, bcol + oc:bcol + oc + 1], None,
                                          OP.add)
            nc.sync.dma_start(qo_t[0:E, :, :], qc_t[64:64 + E, :, :])
            nc.sync.dma_start(ko_t[0:E, :, :], kc_t[64:64 + E, :, :])

            # per-sample attention
            for n in range(RPC):
                cs = slice(n * 128, (n + 1) * 128)
                if phase < 3:
                    break
                # V (+ bias fold) -> v_ext with ones column per head
                pv = psB.tile([128, 512], F32, tag="psB", name="psB")
                for c in range(3):
                    nc.tensor.matmul(pv[:, 0:D], lhsT=xn_fm[:, c, cs], rhs=wv_(c),
                                     start=(c == 0), stop=False)
                nc.tensor.matmul(pv[:, 0:D], lhsT=ones_bf[:, 0:128],
                                 rhs=rl[:, 0:D], start=False, stop=True)
                v_ext = work.tile([128, H, E + 1], BF16, tag="v_ext", name="v_ext")
                nc.vector.memset(v_ext[:, :, E:E + 1], 1.0)
                nc.vector.tensor_copy(
                    v_ext[:, :, 0:E],
                    pv[:, 0:D].rearrange("p (h e) -> p h e", h=H))
                if phase < 4:
                    continue

                # scores S^T = K^T Q per head (all operands base partition 0)
                pss = psA.tile([128, 2, 512], F32, tag="psA", name="psA")
                for h in range(H):
                    kk = kc_t if h % 2 == 0 else ko_t
                    qq = qc_t if h % 2 == 0 else qo_t
                    nc.tensor.matmul(
                        pss[:, h // 4, (h % 4) * 128:(h % 4 + 1) * 128],
                        lhsT=kk[0:E, h // 2, cs],
                        rhs=qq[0:E, h // 2, cs],
                        start=True, stop=True)
                es16 = work.tile([128, H, 128], BF16, tag="es16", name="es16")
                if split_exp:
                    for hf in range(2):
                        nc.scalar.activation(
                            es16[:, hf * 4:(hf + 1) * 4, :],
                            pss[:, hf].rearrange("p (b r) -> p b r", b=4),
                            AF.Exp, scale=TEMP)
                else:
                    nc.scalar.activation(
                        es16[:], pss[:].rearrange("p a (b r) -> p (a b) r", b=4),
                        AF.Exp, scale=TEMP)
                if phase < 5:
                    continue

                # A·[V|1] -> per-head 49-col groups: o unnormalized + row sums
                pso = psB.tile([128, 512], F32, tag="psB", name="psO")
                for h in range(H):
                    nc.tensor.matmul(pso[:, h * 49:(h + 1) * 49],
                                     lhsT=es16[:, h, :], rhs=v_ext[:, h, :],
                                     start=True, stop=True)
                pso_v = pso[:, 0:392].rearrange("p (h e) -> p h e", h=H)
                rr = work.tile([128, H], F32, tag="rr", name="rr")
                nc.vector.reciprocal(rr[:], pso_v[:, :, E])
                o_rm = work.tile([128, D], BF16, tag="o_rm", name="o_rm")
                rrb = AP(rr.tensor, rr.offset, [list(rr.ap[0]), [1, H], [0, E]])
                nc.vector.tensor_tensor(o_rm[:].rearrange("p (h e) -> p h e", h=H),
                                        pso_v[:, :, 0:E], rrb, OP.mult)
                ps = psC.tile([128, 3, 128], BF16, tag="psC", name="psC")
                for c in range(3):
                    nc.tensor.transpose(ps[:, c, :], o_rm[:, c * 128:(c + 1) * 128],
                                        idn_b[:])
                if n % 2:
                    nc.vector.tensor_copy(o_fm[:, :, cs], ps[:])
                else:
                    nc.scalar.copy(o_fm[:, :, cs], ps[:])

            if phase < 6:
                continue

            # LN1: px = o@Wo + bo' + resid(xn)
            def attn_px(rt):
                cs = slice(rt * 128, (rt + 1) * 128)
                px = psB.tile([128, 512], F32, tag="psB", name="psB")
                for c in range(3):
                    nc.tensor.matmul(px[:, 0:D], lhsT=o_fm[:, c, cs], rhs=wo_(c),
                                     start=(c == 0), stop=False)
                nc.tensor.matmul(px[:, 0:D], lhsT=ones_bf[:, 0:128],
                                 rhs=rl[:, D:2 * D], start=False, stop=False)
                if dga is None:
                    nc.tensor.matmul(px[:, 0:D], lhsT=idn_b[:], rhs=xn_rm[:, rt, :],
                                     start=False, stop=True)
                else:
                    dv = dga[:].rearrange("p (c d) -> p c d", c=3)
                    for c in range(3):
                        nc.tensor.matmul(px[:, 0:D], lhsT=xn_fm[:, c, cs],
                                         rhs=dv[:, c], start=False, stop=(c == 2))
                return px

            for h0 in range(0, RPC, LN_HB):
                ln_half(range(h0, h0 + LN_HB), attn_px)

            if phase < 7:
                continue
            # FFN: h1 = relu(xn @ W1' + b1')
            for dc in range(12):
                ph = psA.tile([128, 2, 512], F32, tag="psA", name="psA")
                for hf in range(2):
                    for c in range(3):
                        nc.tensor.matmul(
                            ph[:, hf, :],
                            lhsT=w1_(c)[:, dc * 128:(dc + 1) * 128],
                            rhs=xn_fm[:, c, hf * 512:(hf + 1) * 512],
                            start=(c == 0), stop=(c == 2))
                eng = (nc.scalar, nc.vector)[dc % 2]
                if eng is nc.scalar:
                    nc.scalar.activation(h1[:, dc, :],
                                         ph[:].rearrange("p a b -> p (a b)"),
                                         AF.Relu, bias=bl[:, 8 + dc:9 + dc])
                else:
                    eng.tensor_scalar(h1[:, dc, :],
                                      ph[:].rearrange("p a b -> p (a b)"),
                                      bl[:, 8 + dc:9 + dc], 0.0, OP.add, OP.max)
            if phase < 8:
                continue

            # LN2: px = h1@W2 + b2' + resid(xn)
            def ffn_px(rt):
                cs = slice(rt * 128, (rt + 1) * 128)
                px = psB.tile([128, 512], F32, tag="psB", name="psB")
                for dc in range(12):
                    nc.tensor.matmul(px[:, 0:D], lhsT=h1[:, dc, cs], rhs=w2_(dc),
                                     start=(dc == 0), stop=False)
                nc.tensor.matmul(px[:, 0:D], lhsT=ones_bf[:, 0:128],
                                 rhs=rl[:, 2 * D:3 * D], start=False, stop=False)
                if dgf is None:
                    nc.tensor.matmul(px[:, 0:D], lhsT=idn_b[:], rhs=xn_rm[:, rt, :],
                                     start=False, stop=True)
                else:
                    dv = dgf[:].rearrange("p (c d) -> p c d", c=3)
                    for c in range(3):
                        nc.tensor.matmul(px[:, 0:D], lhsT=xn_fm[:, c, cs],
                                         rhs=dv[:, c], start=False, stop=(c == 2))
                return px

            for h0 in range(0, RPC, LN_HB):
                ln_half(range(h0, h0 + LN_HB), ffn_px)

        # ------------------------------------------------------- head
        outsb = state.tile([1, RPC], F32, tag="outsb", name="outsb")
        for n in range(RPC):
            pm = psC.tile([128, 3], F32, tag="psC", name="psCh")
            for c in range(3):
                nc.tensor.matmul(pm[:, c:c + 1],
                                 lhsT=xn_rm[:, n, c * 128:(c + 1) * 128],
                                 rhs=onesL[:], start=True, stop=True)
            tm = work.tile([128, 3], F32, tag="tm", name="tm")
            nc.scalar.copy(tm[:], pm[:])
            pc = psC.tile([1, 8], F32, tag="psC", name="psCh2")
            for c in range(3):
                nc.tensor.matmul(pc[:, 0:1], lhsT=tm[:, c:c + 1],
                                 rhs=clsw[:, c:c + 1],
                                 start=(c == 0), stop=(c == 2))
            nc.scalar.activation(outsb[:, n:n + 1], pc[:, 0:1], AF.Identity,
                                 bias=clsb[:])
        nc.sync.dma_start(y_d[:].rearrange("a b -> b a"), outsb[:])

    if do_compile:
        nc.compile()
    return nc


_PROG = {}


def _get_prog(gp_ident=None, n_layers=NL, phase=99):
    key = (tuple(gp_ident) if gp_ident else None, n_layers, phase)
    if key not in _PROG:
        _PROG[key] = build_program(gp_ident, n_layers=n_layers, phase=phase)
    return _PROG[key]


def _in_maps(inputs):
    shared = host_prep(inputs)
    gp_ident = shared.pop("gp_identity")
    x = np.asarray(inputs["x"], np.float32)  # (64, 128, 256)
    in_maps = []
    for c in range(NCORES):
        m = dict(shared)
        # [128 tokens, rt, W] per core
        xc = x[c * RPC:(c + 1) * RPC]              # (8, 128, 256)
        m["xc"] = np.ascontiguousarray(xc.transpose(1, 0, 2))
        in_maps.append(m)
    return in_maps, gp_ident


def kernel(**inputs):
    in_maps, gp_ident = _in_maps(inputs)
    nc = _get_prog(gp_ident)
    res = run_bass_kernel_spmd(nc, in_maps, core_ids=list(range(NCORES)))
    out = np.concatenate([res.results[c]["yc"] for c in range(NCORES)], axis=0)
    return out.astype(np.float32)


def timed_run(inputs, iters=32):
    """Estimate per-execution HW time by chaining NEFF executions.

    No NTFF hook is available through this axon tunnel, so true HW exec
    time can't be read from a profile.  Instead we chain k executions
    (each iteration's outputs feed the next call's operands, forcing
    device-side serialization while dispatch pipelines) and report the
    marginal wall time per added execution:  (t_k - t_1) / (k - 1).
    This subtracts the fixed per-dispatch tunnel overhead (~80 ms) that
    would otherwise swamp the measurement.  Returns ns.
    """
    import time
    import jax
    from jax.experimental.shard_map import shard_map
    from jax.sharding import Mesh, NamedSharding, PartitionSpec
    from concourse import bass2jax, mybir as mb

    in_maps, gp_ident = _in_maps(inputs)
    nc = _get_prog(gp_ident)
    bass2jax.install_neuronx_cc_hook()
    partition_name = nc.partition_id_tensor.name if nc.partition_id_tensor else None
    in_names, out_names, out_avals, zero_outs = [], [], [], []
    for alloc in nc.m.functions[0].allocations:
        if not isinstance(alloc, mb.MemoryLocationSet):
            continue
        name = alloc.memorylocations[0].name
        if alloc.kind == "ExternalInput":
            if name != partition_name:
                in_names.append(name)
        elif alloc.kind == "ExternalOutput":
            shape = tuple(alloc.tensor_shape)
            dtype = mb.dt.np(alloc.dtype)
            out_avals.append(jax.core.ShapedArray(shape, dtype))
            out_names.append(name)
            zero_outs.append(np.zeros(shape, dtype))
    n_params, n_outs = len(in_names), len(out_avals)
    all_in = list(in_names) + list(out_names)
    if partition_name is not None:
        all_in.append(partition_name)

    def _body(*args):
        ins = list(args[:n_params])
        outs = list(args[n_params:])
        operands = ins + outs
        if partition_name is not None:
            operands = operands + [bass2jax.partition_id_tensor()]
        outs = list(bass2jax._bass_exec_p.bind(
            *operands, out_avals=tuple(out_avals), in_names=tuple(all_in),
            out_names=tuple(out_names), lowering_input_output_aliases=(),
            sim_require_finite=True, sim_require_nnan=True, nc=nc))
        return tuple(outs)

    devices = jax.devices()[:NCORES]
    mesh = Mesh(np.asarray(devices), ("core",))
    shard = NamedSharding(mesh, PartitionSpec("core"))
    dev_in = [jax.device_put(
        np.concatenate([np.asarray(in_maps[c][nm]) for c in range(NCORES)], axis=0),
        shard) for nm in in_names]
    zsh = [np.zeros((NCORES * z.shape[0], *z.shape[1:]), z.dtype) for z in zero_outs]

    f = jax.jit(
        shard_map(_body, mesh=mesh,
                  in_specs=(PartitionSpec("core"),) * (n_params + n_outs),
                  out_specs=(PartitionSpec("core"),) * n_outs, check_rep=False),
        keep_unused=True)

    def run_chain(k):
        outs = [jax.device_put(z, shard) for z in zsh]
        jax.block_until_ready(outs)
        jax.block_until_ready(dev_in)
        t0 = time.perf_counter()
        for _ in range(k):
            outs = list(f(*dev_in, *outs))
        jax.block_until_ready(outs)
        return time.perf_counter() - t0

    run_chain(1)  # warm compile
    t1 = min(run_chain(1) for _ in range(6))
    tk = min(run_chain(iters) for _ in range(6))
    return int((tk - t1) / (iters - 1) * 1e9)




# revision 6
# speedup vs baseline: 1.9269x; 1.0260x over previous
"""Trainium2 Bass kernel v3 for nn_ClassificationModel.

Data parallel across 8 NeuronCores: batch N=64 -> 8 samples/core.

Differences vs v2 baseline:
  - All per-layer transformer weights packed into ONE [128, 13824] bf16 DMA
    (one HWDGE acquire per layer instead of ~40), double-buffered prefetch.
  - Scores read compact feature-major Q/K via partition-offset lhsT slices
    (heads 2/5 split across chunk boundaries -> 2 accumulating matmuls);
    the per-head 64-row spread DMAs are gone entirely.
  - LayerNorm affine (g, be) folded into adjacent weights host-side; device
    state is the *normalized* activation xn in bf16 (row- and feature-major).
    Residual enters the pre-LN sum as a single identity matmul.  LN stats
    come from accum_out sums (copy + square), apply is one tensor_scalar.
  - 2-PSUM-bank (free=1024) elementwise units for QK bias, scores exp and
    FFN relu; PSUM-consuming ops rotated across ACT/DVE/Pool for balance.
"""

import math
import sys

sys.path.insert(0, "/opt/trn_rl_repo")

import numpy as np
import ml_dtypes

import concourse.bass as bass
import concourse.mybir as mybir
import concourse.tile as tile
from concourse import bacc
from concourse.bass import AP
from concourse.bass_utils import run_bass_kernel_spmd

BF = ml_dtypes.bfloat16
F32 = mybir.dt.float32
BF16 = mybir.dt.bfloat16
AX = mybir.AxisListType
OP = mybir.AluOpType
AF = mybir.ActivationFunctionType

# model dims
N, L, W = 64, 128, 256
D, H, NL, DFF = 384, 8, 4, 1536
E = D // H  # 48
CH = [1, 4, 16, 64]
K = 7
NCORES = 8
RPC = N // NCORES          # samples per core = 8
R = RPC * L                # rows per core = 1024
TEMP = 1.0 / math.sqrt(E)
EPS = 1e-5

# conv block sizes (output positions per Toeplitz block)
B0, B1, B2 = 32, 8, 2
NB0, NB1, NB2 = 256 // B0, 128 // B1, 64 // B2  # 8, 16, 32

# packed per-layer weight blob column offsets (bf16, [128, WCOLS])
# chunk c in 0..2: Wq(512, head-padded) Wk(512) Wv(384) Wo(384) W1(1536)
# at c*3328; then W2: 12 chunks x 384 at 9984.
# Q/K output features are head-padded: head h -> rows 64h..64h+47 of 512,
# so per-head score matmuls read base partitions 0/64 (hw constraint).
CSEC = 512 + 512 + 384 + 384 + 1536  # 3328
PSB_BUFS, PSC_BUFS = 2, 2
LN_HB = 4
WCOLS = 3 * CSEC + 12 * 384          # 14592


# ---------------------------------------------------------------------------
# host-side weight preparation
# ---------------------------------------------------------------------------

def _pe_np(l, d):
    pos = np.arange(l)[:, None].astype(np.float32)
    i = np.arange(d // 2)[None, :].astype(np.float32)
    ang = pos / np.power(10000.0, 2.0 * i / d)
    pe = np.zeros((l, d), np.float32)
    pe[:, 0::2] = np.sin(ang)
    pe[:, 1::2] = np.cos(ang)
    return pe


# conv source-block overlap enumeration (shared host/device) -----------------

CONV_GEOM = {
    0: (B0, 128, 2, 1),
    1: (B1, 32, 4, 4),
    2: (B2, 8, 8, 16),
}


def overlaps(conv, b):
    Bout, src_size, nsrc, _ = CONV_GEOM[conv]
    w0, w1 = Bout * b - 3, Bout * b + Bout + 3
    res = []
    for s in range(nsrc):
        lo, hi = s * src_size, (s + 1) * src_size
        if max(w0, lo) < min(w1, hi):
            res.append((s, lo - Bout * b))
    return res


def conv_deltas(conv):
    nb = {0: NB0, 1: NB1, 2: NB2}[conv]
    return sorted({d for b in range(nb) for _, d in overlaps(conv, b)})


def _m_layout(conv, h, co):
    if conv == 0:
        return (h & 1) * 64 + (h >> 1) * 4 + co
    if conv == 1:
        return (h & 1) * 64 + (h >> 1) * 16 + co
    return h * 64 + co


def _toeplitz_variants(conv, w):
    Bout, src_size, _, nch = CONV_GEOM[conv]
    cout = w.shape[0]
    ds = conv_deltas(conv)
    T = np.zeros((len(ds), src_size * nch, 128), np.float32)
    for vi, delta in enumerate(ds):
        for hp in range(src_size):
            for h in range(Bout):
                k = delta + hp - h + 3
                if 0 <= k < K:
                    for co in range(cout):
                        for ci in range(nch):
                            T[vi, hp * nch + ci, _m_layout(conv, h, co)] = w[co, ci, k]
    return T


def host_prep(inp):
    d = {}
    f32 = np.float32
    d["T0"] = _toeplitz_variants(0, np.asarray(inp["conv_w0"], f32)).astype(BF)
    d["T1"] = _toeplitz_variants(1, np.asarray(inp["conv_w1"], f32)).astype(BF)
    d["T2"] = _toeplitz_variants(2, np.asarray(inp["conv_w2"], f32)).astype(BF)
    b0, b1c, b2c = (np.asarray(inp[f"conv_b{i}"], f32) for i in range(3))
    p = np.arange(128)
    d["b0e"] = b0[p % 4].reshape(128, 1)
    d["b1e"] = b1c[p % 16].reshape(128, 1)
    d["b2e"] = b2c[p % 64].reshape(128, 1)

    # embed: We_r[c, p, :] = embed_w[(p%64)*32 + 2c + p//64, :]
    ew = np.asarray(inp["embed_w"], f32)  # (2048, 384)
    We_r = np.zeros((16, 128, D), f32)
    for c in range(16):
        for pi in range(128):
            We_r[c, pi] = ew[(pi % 64) * 32 + 2 * c + pi // 64]
    d["We_r"] = We_r.astype(BF)
    d["eb_row"] = np.asarray(inp["embed_b"], f32).reshape(1, D).astype(BF)
    d["pe_rm"] = _pe_np(L, D)

    g1 = np.asarray(inp["g1"], f32)
    be1 = np.asarray(inp["be1"], f32)
    g2 = np.asarray(inp["g2"], f32)
    be2 = np.asarray(inp["be2"], f32)

    # pending affine entering each layer's attention block
    gp = np.stack([np.ones(D, f32) if l == 0 else g2[l - 1] for l in range(NL)])
    bp = np.stack([np.zeros(D, f32) if l == 0 else be2[l - 1] for l in range(NL)])
    d["gp_identity"] = [bool(np.all(gp[l] == 1.0)) for l in range(NL)] + \
                       [bool(np.all(g1[l] == 1.0)) for l in range(NL)]

    Wq = np.asarray(inp["Wq"], f32)
    Wk = np.asarray(inp["Wk"], f32)
    Wv = np.asarray(inp["Wv"], f32)
    Wo = np.asarray(inp["Wo"], f32)
    W1 = np.asarray(inp["W1"], f32)
    W2 = np.asarray(inp["W2"], f32)
    bq = np.asarray(inp["bq"], f32)
    bk = np.asarray(inp["bk"], f32)
    bv = np.asarray(inp["bv"], f32)
    bo = np.asarray(inp["bo"], f32)
    b1 = np.asarray(inp["b1"], f32)
    b2 = np.asarray(inp["b2"], f32)

    # fold pending affines into weights/biases
    WqF = gp[:, :, None] * Wq
    WkF = gp[:, :, None] * Wk
    WvF = gp[:, :, None] * Wv
    bqF = bq + np.einsum("ld,lde->le", bp, Wq)
    bkF = bk + np.einsum("ld,lde->le", bp, Wk)
    bvF = bv + np.einsum("ld,lde->le", bp, Wv)
    W1F = g1[:, :, None] * W1
    b1F = b1 + np.einsum("ld,lde->le", be1, W1)
    boF = bo + bp          # LN1 pre-sum bias includes pending be
    b2F = b2 + be1         # LN2 pre-sum bias includes LN1's be

    # diag blocks for non-identity pending g (3 chunks of [128, 384] each)
    def diag_chunks(g):
        out = np.zeros((128, 3 * D), f32)
        for c in range(3):
            for i in range(128):
                out[i, c * D + c * 128 + i] = g[c * 128 + i]
        return out
    d["dg_attn"] = np.stack([diag_chunks(gp[l]) for l in range(NL)]).astype(BF)
    d["dg_ffn"] = np.stack([diag_chunks(g1[l]) for l in range(NL)]).astype(BF)

    # head-pad Q/K output features: head h -> cols 64h..64h+47 of 512
    def head_pad_w(w):  # (NL, 384, 384) -> (NL, 384, 512)
        out = np.zeros((NL, D, 512), f32)
        for h in range(H):
            out[:, :, 64 * h:64 * h + E] = w[:, :, E * h:E * (h + 1)]
        return out

    def head_pad_b(b):  # (NL, 384) -> (NL, 512)
        out = np.zeros((NL, 512), f32)
        for h in range(H):
            out[:, 64 * h:64 * h + E] = b[:, E * h:E * (h + 1)]
        return out

    WqP, WkP = head_pad_w(WqF), head_pad_w(WkF)
    bqP, bkP = head_pad_b(bqF), head_pad_b(bkF)

    # mega weight blob per layer
    WL = np.zeros((NL, 128, WCOLS), f32)
    for l in range(NL):
        for c in range(3):
            r = slice(c * 128, (c + 1) * 128)
            base = c * CSEC
            WL[l, :, base + 0:base + 512] = WqP[l][r]
            WL[l, :, base + 512:base + 1024] = WkP[l][r]
            WL[l, :, base + 1024:base + 1408] = WvF[l][r]
            WL[l, :, base + 1408:base + 1792] = Wo[l][r]
            WL[l, :, base + 1792:base + 3328] = W1F[l][r]
        for dc in range(12):
            WL[l, :, 3 * CSEC + dc * 384:3 * CSEC + (dc + 1) * 384] = \
                W2[l][dc * 128:(dc + 1) * 128]
    d["WL"] = WL.astype(BF)

    # per-layer f32 bias blob [128, 20]: bq (4 padded chunks), bk, b1r (12)
    BL = np.zeros((NL, 128, 20), f32)
    for oc in range(4):
        BL[:, :, oc] = bqP[:, oc * 128:(oc + 1) * 128]
        BL[:, :, 4 + oc] = bkP[:, oc * 128:(oc + 1) * 128]
    for l in range(NL):
        BL[l, :, 8:20] = b1F[l].reshape(12, 128).T
    d["BL"] = BL

    # per-layer bf16 rows blob [1, 1152]: bv | bo' | b2'
    RL = np.zeros((NL, 1, 3 * D), f32)
    RL[:, 0, 0:D] = bvF
    RL[:, 0, D:2 * D] = boF
    RL[:, 0, 2 * D:3 * D] = b2F
    d["RL"] = RL.astype(BF)

    d["idn_f"] = np.eye(128, dtype=f32)
    d["idn_b"] = np.eye(128, dtype=f32).astype(BF)
    d["onesL"] = np.full((128, 1), 1.0 / L, f32).astype(BF)
    # head fold: mean(g2[3]*xn + be2[3]) @ cls_w + cls_b
    cw = np.asarray(inp["cls_w"], f32)          # (384, 1)
    cb = np.asarray(inp["cls_b"], f32)          # (1,)
    cwF = (g2[NL - 1][:, None] * cw)
    cbF = cb + be2[NL - 1] @ cw
    d["clsw_r"] = cwF.reshape(3, 128).T.copy()  # (128, 3)
    d["clsb"] = cbF.reshape(1, 1)
    d["epsc"] = np.full((128, 1), EPS, f32)
    return d


# ---------------------------------------------------------------------------
# device program
# ---------------------------------------------------------------------------

def build_program(gp_ident=None, do_compile=True, n_layers=NL, phase=99, split_exp=False, pad_scores=False, even_only=False):
    if gp_ident is None:
        gp_ident = [True] * (2 * NL)
    nc = bacc.Bacc("TRN2", target_bir_lowering=False, debug=False)

    def dram_in(name, shape, dt=BF16):
        return nc.dram_tensor(name, list(shape), dt, kind="ExternalInput")

    x_d = dram_in("xc", (128, RPC, W), F32)
    nv0, nv1, nv2 = len(conv_deltas(0)), len(conv_deltas(1)), len(conv_deltas(2))
    T0_d = dram_in("T0", (nv0, 128, 128))
    T1_d = dram_in("T1", (nv1, 128, 128))
    T2_d = dram_in("T2", (nv2, 128, 128))
    b0e_d = dram_in("b0e", (128, 1), F32)
    b1e_d = dram_in("b1e", (128, 1), F32)
    b2e_d = dram_in("b2e", (128, 1), F32)
    We_d = dram_in("We_r", (16, 128, D))
    ebr_d = dram_in("eb_row", (1, D))
    pe_d = dram_in("pe_rm", (128, D), F32)
    WL_d = dram_in("WL", (NL, 128, WCOLS))
    BL_d = dram_in("BL", (NL, 128, 20), F32)
    RL_d = dram_in("RL", (NL, 1, 3 * D))
    dga_d = dram_in("dg_attn", (NL, 128, 3 * D))
    dgf_d = dram_in("dg_ffn", (NL, 128, 3 * D))
    idnf_d = dram_in("idn_f", (128, 128), F32)
    idnb_d = dram_in("idn_b", (128, 128))
    onesL_d = dram_in("onesL", (128, 1))
    clsw_d = dram_in("clsw_r", (128, 3), F32)
    eps_d = dram_in("epsc", (128, 1), F32)
    clsb_d = dram_in("clsb", (1, 1), F32)

    y_d = nc.dram_tensor("yc", [RPC, 1], F32, kind="ExternalOutput")

    from contextlib import ExitStack
    with tile.TileContext(nc) as tc, ExitStack() as ctx:
        const = ctx.enter_context(tc.tile_pool(name="const", bufs=1))
        state = ctx.enter_context(tc.tile_pool(name="state", bufs=1))
        psA = ctx.enter_context(tc.tile_pool(name="psA", bufs=2, space="PSUM"))
        psB = ctx.enter_context(tc.tile_pool(name="psB", bufs=PSB_BUFS, space="PSUM"))
        psC = ctx.enter_context(tc.tile_pool(name="psC", bufs=PSC_BUFS, space="PSUM"))

        # full input in one DMA (CNN-scoped pool, released before transformer)
        def load_const_in(pool, dram, shape, dt):
            nm = dram.name + "_sb"
            t = pool.tile(list(shape), dt, tag=nm, name=nm)
            nc.sync.dma_start(t[:], dram[:])
            return t

        const = ctx.enter_context(tc.tile_pool(name="const", bufs=1))
        state = ctx.enter_context(tc.tile_pool(name="state", bufs=1))
        idn_f = load_const_in(const, idnf_d, (128, 128), F32)
        idn_b = load_const_in(const, idnb_d, (128, 128), BF16)
        onesL = load_const_in(const, onesL_d, (128, 1), BF16)
        clsw = load_const_in(const, clsw_d, (128, 3), F32)
        epsc = load_const_in(const, eps_d, (128, 1), F32)
        clsb = load_const_in(const, clsb_d, (1, 1), F32)
        ones_bf = const.tile([1, 512], BF16, tag="ones_bf", name="ones_bf")
        nc.vector.memset(ones_bf[:], 1.0)

        # persistent state written by CNN: normalized activations, bf16
        xn_rm = state.tile([128, RPC, D], BF16, tag="xn_rm", name="xn_rm")
        xn_fm = state.tile([128, 3, R], BF16, tag="xn_fm", name="xn_fm")

        # transformer weights pool must outlive the CNN block (prefetch L0)
        wpool = ctx.enter_context(tc.tile_pool(name="wpool", bufs=2))

        WLs, BLs, RLs, DGAs, DGFs = [], [], [], [], []
        def load_layer(l):
            wl = wpool.tile([128, WCOLS], BF16, tag="WL", name=f"WL{l}")
            nc.sync.dma_start(wl[:], WL_d[l])
            bl = wpool.tile([128, 20], F32, tag="BL", name=f"BL{l}")
            nc.sync.dma_start(bl[:], BL_d[l])
            rl = wpool.tile([1, 3 * D], BF16, tag="RL", name=f"RL{l}")
            nc.sync.dma_start(rl[:], RL_d[l])
            dga = dgf = None
            if not gp_ident[l]:
                dga = wpool.tile([128, 3 * D], BF16, tag="DGA", name=f"DGA{l}")
                nc.sync.dma_start(dga[:], dga_d[l])
            if not gp_ident[NL + l]:
                dgf = wpool.tile([128, 3 * D], BF16, tag="DGF", name=f"DGF{l}")
                nc.sync.dma_start(dgf[:], dgf_d[l])
            return (wl, bl, rl, dga, dgf)

        cur = load_layer(0)

        # ------------------------------------------------------- CNN + embed
        # 4 row-tiles per group: conv matmuls move 512 cols (4 rts) at once
        with tc.tile_pool(name="cnnc", bufs=1) as cnnc, \
                tc.tile_pool(name="cnn", bufs=2) as cnnp:
            x_all = cnnc.tile([128, RPC, W], F32, tag="x_all", name="x_all")
            nc.sync.dma_start(x_all[:], x_d[:])
            T0v, T1v, T2v = [], [], []
            for conv, (dst, dram, npart) in enumerate(
                    ((T0v, T0_d, 128), (T1v, T1_d, 128), (T2v, T2_d, 128))):
                for vi in range(len(conv_deltas(conv))):
                    t = cnnc.tile([npart, 128], BF16, tag=f"Tv{conv}_{vi}",
                                  name=f"Tv{conv}_{vi}")
                    nc.sync.dma_start(t[:], dram[vi])
                    dst.append(t)
            d2i = [{d: i for i, d in enumerate(conv_deltas(c))} for c in range(3)]
            b0e = load_const_in(cnnc, b0e_d, (128, 1), F32)
            b1e = load_const_in(cnnc, b1e_d, (128, 1), F32)
            b2e = load_const_in(cnnc, b2e_d, (128, 1), F32)
            eb_row = load_const_in(cnnc, ebr_d, (1, D), BF16)
            pe_rm = load_const_in(cnnc, pe_d, (128, D), F32)
            We = []
            for c in range(16):
                t = cnnc.tile([128, D], BF16, tag=f"We{c}", name=f"We{c}")
                nc.sync.dma_start(t[:], We_d[c])
                We.append(t)

            for g in range(2):
                rts = range(g * 4, (g + 1) * 4)
                # transpose x: per rt, both halves -> xt4 [128, half, rt, 128]
                xt4 = cnnp.tile([128, 2, 4, 128], BF16, tag="xt4", name="xt4")
                for j, rt in enumerate(rts):
                    psx = psC.tile([128, 3, 128], F32, tag="psC", name="psC")
                    for half in range(2):
                        nc.tensor.transpose(
                            psx[:, half, :],
                            x_all[:, rt, half * 128:(half + 1) * 128], idn_f[:])
                    if j % 2 == 0:
                        nc.scalar.copy(xt4[:, :, j, :], psx[:, 0:2, :])
                    else:
                        nc.vector.tensor_copy(xt4[:, :, j, :], psx[:, 0:2, :])

                def conv_unit(conv, Tv, srcs, bias, b0, out_cb):
                    """blocks b0, b0+1 x 4 rts -> one 2-bank psum; hi-half
                    relu+bias on ACT -> r_hi; out_cb(ps, r_hi) pools."""
                    ps = psA.tile([128, 2, 512], F32, tag="psA", name="psA")
                    for bi in range(2):
                        ovl = overlaps(conv, b0 + bi)
                        for i, (s, dlt) in enumerate(ovl):
                            nc.tensor.matmul(
                                ps[:, bi, :],
                                lhsT=Tv[d2i[conv][dlt]][:], rhs=srcs(s),
                                start=(i == 0), stop=(i == len(ovl) - 1))
                    r_hi = cnnp.tile([64, 2, 512], BF16, tag="r_hi", name="r_hi")
                    nc.scalar.activation(r_hi[:], ps[64:128], AF.Relu,
                                         bias=bias[64:128, :])
                    out_cb(ps, r_hi)

                # conv0 -> pooled0 [128, 4, 4, 128]: block b's 16 pooled
                # positions -> partitions 64*(b%2)+4j+ci of tile b//2
                pooled0 = cnnp.tile([128, NB0 // 2, 4, 128], BF16, tag="pooled0",
                                    name="pooled0")
                for b0_ in range(0, NB0, 2):
                    def p0(ps, r_hi, b0_=b0_):
                        for bi in range(2):
                            nc.vector.scalar_tensor_tensor(
                                pooled0[64 * bi:64 * bi + 64, b0_ // 2, :, :],
                                in0=ps[0:64, bi].rearrange(
                                    "p (j r) -> p j r", j=4),
                                scalar=b0e[0:64, :],
                                in1=r_hi[:, bi].rearrange(
                                    "p (j r) -> p j r", j=4),
                                op0=OP.add, op1=OP.max)
                    conv_unit(0, T0v, lambda s: xt4[:, s, :, :], b0e, b0_, p0)

                # conv1 -> pooled1 [128, 8, 4, 128]: conv1 block b's 4
                # pooled positions land at partitions 64*(b%2)+j*16+ci of
                # tile b//2 (8-pos/16-ch source tiles for conv2)
                pooled1 = cnnp.tile([128, NB1 // 2, 4, 128], BF16, tag="pooled1",
                                    name="pooled1")
                for b0_ in range(0, NB1, 2):
                    def p1(ps, r_hi, b0_=b0_):
                        for bi in range(2):
                            nc.vector.scalar_tensor_tensor(
                                pooled1[64 * bi:64 * bi + 64,
                                        b0_ // 2, :, :],
                                in0=ps[0:64, bi].rearrange(
                                    "p (j r) -> p j r", j=4),
                                scalar=b1e[0:64, :],
                                in1=r_hi[:, bi].rearrange(
                                    "p (j r) -> p j r", j=4),
                                op0=OP.add, op1=OP.max)
                    conv_unit(1, T1v, lambda s: pooled0[:, s, :, :], b1e, b0_, p1)

                # conv2 -> act3 [128, 16, 4, 128]; parity -> partition half
                act3 = cnnp.tile([128, 16, 4, 128], BF16, tag="act3", name="act3")
                for b0_ in range(0, NB2, 2):
                    def p2(ps, r_hi, b0_=b0_):
                        ch = b0_ // 2
                        nc.vector.scalar_tensor_tensor(
                            act3[0:64, ch, :, :],
                            in0=ps[0:64, 0].rearrange("p (j r) -> p j r", j=4),
                            scalar=b2e[0:64, :],
                            in1=r_hi[:, 0].rearrange("p (j r) -> p j r", j=4),
                            op0=OP.add, op1=OP.max)
                        nc.vector.scalar_tensor_tensor(
                            act3[64:128, ch, :, :],
                            in0=ps[0:64, 1].rearrange("p (j r) -> p j r", j=4),
                            scalar=b2e[0:64, :],
                            in1=r_hi[:, 1].rearrange("p (j r) -> p j r", j=4),
                            op0=OP.add, op1=OP.max)
                    conv_unit(2, T2v, lambda s: pooled1[:, s, :, :], b2e, b0_, p2)

                # embed + bias + relu + pe -> xn_rm / xn_fm per rt
                for j, rt in enumerate(rts):
                    pse = psB.tile([128, 512], F32, tag="psB", name="psB")
                    for c in range(16):
                        nc.tensor.matmul(pse[:, 0:D], lhsT=act3[:, c, j, :],
                                         rhs=We[c][:],
                                         start=(c == 0), stop=False)
                    nc.tensor.matmul(pse[:, 0:D], lhsT=ones_bf[:, 0:128],
                                     rhs=eb_row[:], start=False, stop=True)
                    nc.vector.scalar_tensor_tensor(
                        xn_rm[:, rt, :], in0=pse[:, 0:D], scalar=0.0,
                        in1=pe_rm[:], op0=OP.max, op1=OP.add)
                    psx = psC.tile([128, 3, 128], BF16, tag="psC", name="psC2")
                    for c in range(3):
                        nc.tensor.transpose(psx[:, c, :],
                                            xn_rm[:, rt, c * 128:(c + 1) * 128],
                                            idn_b[:])
                    if rt % 2:
                        nc.vector.tensor_copy(
                            xn_fm[:, :, rt * 128:(rt + 1) * 128], psx[:])
                    else:
                        nc.scalar.copy(xn_fm[:, :, rt * 128:(rt + 1) * 128],
                                       psx[:])

        # transformer-only state (own pool: allocated after CNN pools
        # release so it reuses their SBUF space)
        tstate = ctx.enter_context(tc.tile_pool(name="tstate", bufs=1))
        o_fm = tstate.tile([128, 3, R], BF16, tag="o_fm", name="o_fm")
        h1 = tstate.tile([128, 12, R], BF16, tag="h1", name="h1")
        qc_t = tstate.tile([128, 4, R], BF16, tag="qc_t", name="qc_t")
        kc_t = tstate.tile([128, 4, R], BF16, tag="kc_t", name="kc_t")
        qo_t = tstate.tile([64, 4, R], BF16, tag="qo_t", name="qo_t")
        ko_t = tstate.tile([64, 4, R], BF16, tag="ko_t", name="ko_t")

        # ------------------------------------------------------- transformer
        work = ctx.enter_context(tc.tile_pool(name="work", bufs=3))
        lnw = ctx.enter_context(tc.tile_pool(name="lnw", bufs=2))

        def ln_half(rts, px_of, write_fm=True):
            """Half-batch layernorm: for rts, px_of(rt) emits matmuls into a
            fresh psB and returns it (pre-LN sum incl. residual+bias).
            Writes xn_rm / xn_fm."""
            x1s = {}
            st = lnw.tile([128, 8, 2], F32, tag="st", name="st")  # s1, s2
            for j, rt in enumerate(rts):
                px = px_of(rt)
                x1 = lnw.tile([128, D], BF16, tag=f"x1_{j}", name=f"x1_{j}")
                # copy + running sum  (rotate ACT / Pool)
                if j % 2 == 0:
                    nc.scalar.activation(x1[:], px[:, 0:D], AF.Identity,
                                         accum_out=st[:, j, 0:1])
                else:
                    nc.vector.tensor_scalar(x1[:], px[:, 0:D], 0.0, 0.0, OP.add,
                                            OP.add, accum_out=st[:, j, 0:1])
                xsq = work.tile([128, D], BF16, tag="xsq", name="xsq")
                nc.vector.scalar_tensor_tensor(
                    xsq[:], in0=x1[:], scalar=0.0, in1=x1[:],
                    op0=OP.add, op1=OP.mult, accum_out=st[:, j, 1:2])
                x1s[rt] = x1
            # mean = s1/D; var = s2/D - mean^2 ; rstd = 1/sqrt(var+eps)
            nb = len(rts)
            mean = lnw.tile([128, 8], F32, tag="mean", name="mean")
            var = lnw.tile([128, 8], F32, tag="var", name="var")
            nc.vector.tensor_scalar(mean[:, 0:nb], st[:, 0:nb, 0], 1.0 / D, None,
                                    OP.mult)
            msq = work.tile([128, 8], F32, tag="msq", name="msq")
            nc.vector.tensor_tensor(msq[:, 0:nb], mean[:, 0:nb], mean[:, 0:nb],
                                    OP.mult)
            nc.vector.scalar_tensor_tensor(
                var[:, 0:nb], in0=st[:, 0:nb, 1], scalar=1.0 / D, in1=msq[:, 0:nb],
                op0=OP.mult, op1=OP.subtract)
            sd = work.tile([128, 8], F32, tag="sd", name="sd")
            nc.scalar.activation(sd[:, 0:nb], var[:, 0:nb], AF.Sqrt, bias=epsc[:])
            rstd = lnw.tile([128, 8], F32, tag="rstd", name="rstd")
            nc.vector.reciprocal(rstd[:, 0:nb], sd[:, 0:nb])
            mrs = lnw.tile([128, 8], F32, tag="mrs", name="mrs")
            nc.vector.scalar_tensor_tensor(
                mrs[:, 0:nb], in0=mean[:, 0:nb], scalar=-1.0, in1=rstd[:, 0:nb],
                op0=OP.mult, op1=OP.mult)
            for j, rt in enumerate(rts):
                nc.vector.tensor_scalar(
                    xn_rm[:, rt, :], x1s[rt][:], rstd[:, j:j + 1],
                    mrs[:, j:j + 1], OP.mult, OP.add)
                if not write_fm:
                    continue
                pst = psC.tile([128, 3, 128], BF16, tag="psC", name="psC")
                for c in range(3):
                    nc.tensor.transpose(pst[:, c, :],
                                        xn_rm[:, rt, c * 128:(c + 1) * 128],
                                        idn_b[:])
                if j % 2 == 0:
                    nc.vector.tensor_copy(
                        xn_fm[:, :, rt * 128:(rt + 1) * 128], pst[:])
                else:
                    nc.scalar.copy(xn_fm[:, :, rt * 128:(rt + 1) * 128], pst[:])

        for lyr in range(n_layers):
            wl, bl, rl, dga, dgf = cur
            if lyr + 1 < n_layers:
                cur = load_layer(lyr + 1)

            def wv_(c):   # [128, 384] views into the blob
                return wl[:, c * CSEC + 1024:c * CSEC + 1408]
            def wo_(c):
                return wl[:, c * CSEC + 1408:c * CSEC + 1792]
            def w1_(c):
                return wl[:, c * CSEC + 1792:c * CSEC + 3328]
            def w2_(dc):
                return wl[:, 3 * CSEC + dc * 384:3 * CSEC + (dc + 1) * 384]

            if phase < 2:
                continue
            # batched Q/K into head-padded feature-major tiles (4 out-chunks)
            for pi, (dstc, woff, bcol) in enumerate(
                    ((qc_t, 0, 0), (kc_t, 512, 4))):
                for oc in range(4):
                    pq = psA.tile([128, 2, 512], F32, tag="psA", name="psA")
                    for hf in range(2):
                        for c in range(3):
                            nc.tensor.matmul(
                                pq[:, hf, :],
                                lhsT=wl[:, c * CSEC + woff + oc * 128:
                                        c * CSEC + woff + (oc + 1) * 128],
                                rhs=xn_fm[:, c, hf * 512:(hf + 1) * 512],
                                start=(c == 0), stop=(c == 2))
                    eng = (nc.scalar, nc.vector)[(pi * 4 + oc) % 2]
                    if eng is nc.scalar:
                        nc.scalar.activation(dstc[:, oc, :],
                                             pq[:].rearrange("p a b -> p (a b)"),
                                             AF.Identity,
                                             bias=bl[:, bcol + oc:bcol + oc + 1])
                    else:
                        eng.tensor_scalar(dstc[:, oc, :],
                                          pq[:].rearrange("p a b -> p (a b)"),
                                          bl[:, bcol + oc:bcol + oc + 1], None,
                                          OP.add)
            nc.sync.dma_start(qo_t[0:E, :, :], qc_t[64:64 + E, :, :])
            nc.sync.dma_start(ko_t[0:E, :, :], kc_t[64:64 + E, :, :])

            # per-sample attention
            for n in range(RPC):
                cs = slice(n * 128, (n + 1) * 128)
                if phase < 3:
                    break
                # V (+ bias fold) -> v_ext with ones column per head
                pv = psB.tile([128, 512], F32, tag="psB", name="psB")
                for c in range(3):
                    nc.tensor.matmul(pv[:, 0:D], lhsT=xn_fm[:, c, cs], rhs=wv_(c),
                                     start=(c == 0), stop=False)
                nc.tensor.matmul(pv[:, 0:D], lhsT=ones_bf[:, 0:128],
                                 rhs=rl[:, 0:D], start=False, stop=True)
                v_ext = work.tile([128, H, E + 1], BF16, tag="v_ext", name="v_ext")
                nc.vector.memset(v_ext[:, :, E:E + 1], 1.0)
                nc.vector.tensor_copy(
                    v_ext[:, :, 0:E],
                    pv[:, 0:D].rearrange("p (h e) -> p h e", h=H))
                if phase < 4:
                    continue

                # scores S^T = K^T Q per head (all operands base partition 0)
                pss = psA.tile([128, 2, 512], F32, tag="psA", name="psA")
                for h in range(H):
                    kk = kc_t if h % 2 == 0 else ko_t
                    qq = qc_t if h % 2 == 0 else qo_t
                    nc.tensor.matmul(
                        pss[:, h // 4, (h % 4) * 128:(h % 4 + 1) * 128],
                        lhsT=kk[0:E, h // 2, cs],
                        rhs=qq[0:E, h // 2, cs],
                        start=True, stop=True)
                es16 = work.tile([128, H, 128], BF16, tag="es16", name="es16")
                if split_exp:
                    for hf in range(2):
                        nc.scalar.activation(
                            es16[:, hf * 4:(hf + 1) * 4, :],
                            pss[:, hf].rearrange("p (b r) -> p b r", b=4),
                            AF.Exp, scale=TEMP)
                else:
                    nc.scalar.activation(
                        es16[:], pss[:].rearrange("p a (b r) -> p (a b) r", b=4),
                        AF.Exp, scale=TEMP)
                if phase < 5:
                    continue

                # A·[V|1] -> per-head 49-col groups: o unnormalized + row sums
                pso = psB.tile([128, 512], F32, tag="psB", name="psO")
                for h in range(H):
                    nc.tensor.matmul(pso[:, h * 49:(h + 1) * 49],
                                     lhsT=es16[:, h, :], rhs=v_ext[:, h, :],
                                     start=True, stop=True)
                pso_v = pso[:, 0:392].rearrange("p (h e) -> p h e", h=H)
                rr = work.tile([128, H], F32, tag="rr", name="rr")
                nc.vector.reciprocal(rr[:], pso_v[:, :, E])
                o_rm = work.tile([128, D], BF16, tag="o_rm", name="o_rm")
                rrb = AP(rr.tensor, rr.offset, [list(rr.ap[0]), [1, H], [0, E]])
                nc.vector.tensor_tensor(o_rm[:].rearrange("p (h e) -> p h e", h=H),
                                        pso_v[:, :, 0:E], rrb, OP.mult)
                ps = psC.tile([128, 3, 128], BF16, tag="psC", name="psC")
                for c in range(3):
                    nc.tensor.transpose(ps[:, c, :], o_rm[:, c * 128:(c + 1) * 128],
                                        idn_b[:])
                if n % 2:
                    nc.vector.tensor_copy(o_fm[:, :, cs], ps[:])
                else:
                    nc.scalar.copy(o_fm[:, :, cs], ps[:])

            if phase < 6:
                continue

            # LN1: px = o@Wo + bo' + resid(xn)
            def attn_px(rt):
                cs = slice(rt * 128, (rt + 1) * 128)
                px = psB.tile([128, 512], F32, tag="psB", name="psB")
                for c in range(3):
                    nc.tensor.matmul(px[:, 0:D], lhsT=o_fm[:, c, cs], rhs=wo_(c),
                                     start=(c == 0), stop=False)
                nc.tensor.matmul(px[:, 0:D], lhsT=ones_bf[:, 0:128],
                                 rhs=rl[:, D:2 * D], start=False, stop=False)
                if dga is None:
                    nc.tensor.matmul(px[:, 0:D], lhsT=idn_b[:], rhs=xn_rm[:, rt, :],
                                     start=False, stop=True)
                else:
                    dv = dga[:].rearrange("p (c d) -> p c d", c=3)
                    for c in range(3):
                        nc.tensor.matmul(px[:, 0:D], lhsT=xn_fm[:, c, cs],
                                         rhs=dv[:, c], start=False, stop=(c == 2))
                return px

            for h0 in range(0, RPC, LN_HB):
                ln_half(range(h0, h0 + LN_HB), attn_px)

            if phase < 7:
                continue
            # FFN: h1 = relu(xn @ W1' + b1')
            for dc in range(12):
                ph = psA.tile([128, 2, 512], F32, tag="psA", name="psA")
                for hf in range(2):
                    for c in range(3):
                        nc.tensor.matmul(
                            ph[:, hf, :],
                            lhsT=w1_(c)[:, dc * 128:(dc + 1) * 128],
                            rhs=xn_fm[:, c, hf * 512:(hf + 1) * 512],
                            start=(c == 0), stop=(c == 2))
                eng = (nc.scalar, nc.vector)[dc % 2]
                if eng is nc.scalar:
                    nc.scalar.activation(h1[:, dc, :],
                                         ph[:].rearrange("p a b -> p (a b)"),
                                         AF.Relu, bias=bl[:, 8 + dc:9 + dc])
                else:
                    eng.tensor_scalar(h1[:, dc, :],
                                      ph[:].rearrange("p a b -> p (a b)"),
                                      bl[:, 8 + dc:9 + dc], 0.0, OP.add, OP.max)
            if phase < 8:
                continue

            # LN2: px = h1@W2 + b2' + resid(xn)
            def ffn_px(rt):
                cs = slice(rt * 128, (rt + 1) * 128)
                px = psB.tile([128, 512], F32, tag="psB", name="psB")
                for dc in range(12):
                    nc.tensor.matmul(px[:, 0:D], lhsT=h1[:, dc, cs], rhs=w2_(dc),
                                     start=(dc == 0), stop=False)
                nc.tensor.matmul(px[:, 0:D], lhsT=ones_bf[:, 0:128],
                                 rhs=rl[:, 2 * D:3 * D], start=False, stop=False)
                if dgf is None:
                    nc.tensor.matmul(px[:, 0:D], lhsT=idn_b[:], rhs=xn_rm[:, rt, :],
                                     start=False, stop=True)
                else:
                    dv = dgf[:].rearrange("p (c d) -> p c d", c=3)
                    for c in range(3):
                        nc.tensor.matmul(px[:, 0:D], lhsT=xn_fm[:, c, cs],
                                         rhs=dv[:, c], start=False, stop=(c == 2))
                return px

            last = (lyr == n_layers - 1) and gp_ident[NL + lyr]
            for h0 in range(0, RPC, LN_HB):
                ln_half(range(h0, h0 + LN_HB), ffn_px, write_fm=not last)

        # ------------------------------------------------------- head
        outsb = state.tile([1, RPC], F32, tag="outsb", name="outsb")
        for n in range(RPC):
            pm = psC.tile([128, 3], F32, tag="psC", name="psCh")
            for c in range(3):
                nc.tensor.matmul(pm[:, c:c + 1],
                                 lhsT=xn_rm[:, n, c * 128:(c + 1) * 128],
                                 rhs=onesL[:], start=True, stop=True)
            tm = work.tile([128, 3], F32, tag="tm", name="tm")
            nc.scalar.copy(tm[:], pm[:])
            pc = psC.tile([1, 8], F32, tag="psC", name="psCh2")
            for c in range(3):
                nc.tensor.matmul(pc[:, 0:1], lhsT=tm[:, c:c + 1],
                                 rhs=clsw[:, c:c + 1],
                                 start=(c == 0), stop=(c == 2))
            nc.scalar.activation(outsb[:, n:n + 1], pc[:, 0:1], AF.Identity,
                                 bias=clsb[:])
        nc.sync.dma_start(y_d[:].rearrange("a b -> b a"), outsb[:])

    if do_compile:
        nc.compile()
    return nc


_PROG = {}


def _get_prog(gp_ident=None, n_layers=NL, phase=99):
    key = (tuple(gp_ident) if gp_ident else None, n_layers, phase)
    if key not in _PROG:
        _PROG[key] = build_program(gp_ident, n_layers=n_layers, phase=phase)
    return _PROG[key]


def _in_maps(inputs):
    shared = host_prep(inputs)
    gp_ident = shared.pop("gp_identity")
    x = np.asarray(inputs["x"], np.float32)  # (64, 128, 256)
    in_maps = []
    for c in range(NCORES):
        m = dict(shared)
        # [128 tokens, rt, W] per core
        xc = x[c * RPC:(c + 1) * RPC]              # (8, 128, 256)
        m["xc"] = np.ascontiguousarray(xc.transpose(1, 0, 2))
        in_maps.append(m)
    return in_maps, gp_ident


def kernel(**inputs):
    in_maps, gp_ident = _in_maps(inputs)
    nc = _get_prog(gp_ident)
    res = run_bass_kernel_spmd(nc, in_maps, core_ids=list(range(NCORES)))
    out = np.concatenate([res.results[c]["yc"] for c in range(NCORES)], axis=0)
    return out.astype(np.float32)


def timed_run(inputs, iters=32):
    """Estimate per-execution HW time by chaining NEFF executions.

    No NTFF hook is available through this axon tunnel, so true HW exec
    time can't be read from a profile.  Instead we chain k executions
    (each iteration's outputs feed the next call's operands, forcing
    device-side serialization while dispatch pipelines) and report the
    marginal wall time per added execution:  (t_k - t_1) / (k - 1).
    This subtracts the fixed per-dispatch tunnel overhead (~80 ms) that
    would otherwise swamp the measurement.  Returns ns.
    """
    import time
    import jax
    from jax.experimental.shard_map import shard_map
    from jax.sharding import Mesh, NamedSharding, PartitionSpec
    from concourse import bass2jax, mybir as mb

    in_maps, gp_ident = _in_maps(inputs)
    nc = _get_prog(gp_ident)
    bass2jax.install_neuronx_cc_hook()
    partition_name = nc.partition_id_tensor.name if nc.partition_id_tensor else None
    in_names, out_names, out_avals, zero_outs = [], [], [], []
    for alloc in nc.m.functions[0].allocations:
        if not isinstance(alloc, mb.MemoryLocationSet):
            continue
        name = alloc.memorylocations[0].name
        if alloc.kind == "ExternalInput":
            if name != partition_name:
                in_names.append(name)
        elif alloc.kind == "ExternalOutput":
            shape = tuple(alloc.tensor_shape)
            dtype = mb.dt.np(alloc.dtype)
            out_avals.append(jax.core.ShapedArray(shape, dtype))
            out_names.append(name)
            zero_outs.append(np.zeros(shape, dtype))
    n_params, n_outs = len(in_names), len(out_avals)
    all_in = list(in_names) + list(out_names)
    if partition_name is not None:
        all_in.append(partition_name)

    def _body(*args):
        ins = list(args[:n_params])
        outs = list(args[n_params:])
        operands = ins + outs
        if partition_name is not None:
            operands = operands + [bass2jax.partition_id_tensor()]
        outs = list(bass2jax._bass_exec_p.bind(
            *operands, out_avals=tuple(out_avals), in_names=tuple(all_in),
            out_names=tuple(out_names), lowering_input_output_aliases=(),
            sim_require_finite=True, sim_require_nnan=True, nc=nc))
        return tuple(outs)

    devices = jax.devices()[:NCORES]
    mesh = Mesh(np.asarray(devices), ("core",))
    shard = NamedSharding(mesh, PartitionSpec("core"))
    dev_in = [jax.device_put(
        np.concatenate([np.asarray(in_maps[c][nm]) for c in range(NCORES)], axis=0),
        shard) for nm in in_names]
    zsh = [np.zeros((NCORES * z.shape[0], *z.shape[1:]), z.dtype) for z in zero_outs]

    f = jax.jit(
        shard_map(_body, mesh=mesh,
                  in_specs=(PartitionSpec("core"),) * (n_params + n_outs),
                  out_specs=(PartitionSpec("core"),) * n_outs, check_rep=False),
        keep_unused=True)

    def run_chain(k):
        outs = [jax.device_put(z, shard) for z in zsh]
        jax.block_until_ready(outs)
        jax.block_until_ready(dev_in)
        t0 = time.perf_counter()
        for _ in range(k):
            outs = list(f(*dev_in, *outs))
        jax.block_until_ready(outs)
        return time.perf_counter() - t0

    run_chain(1)  # warm compile
    t1 = min(run_chain(1) for _ in range(6))
    tk = min(run_chain(iters) for _ in range(6))
    return int((tk - t1) / (iters - 1) * 1e9)


